# revision 32
# baseline (speedup 1.0000x reference)
"""Multi-head attention kernel for Trainium2, sharded over 8 NeuronCores.

Problem: B=2, S=2048, HIDDEN=1024, 16 heads, head_dim=64, fp32 in/out.

Sharding (data + tensor parallel per the hint): core c handles batch b=c//4
and head-group g=c%4 (4 heads = 256 hidden columns). QKV projections are
column-sharded, output projection row-sharded; each core returns a partial
out^T [1024, 2048] and the host sums the 4 partials per batch (the
row-parallel all-reduce) and transposes.

All matmul operands are bf16 (fp32 matmul runs at 1/4 PE rate: 2 half-speed
passes); accumulation is fp32 in PSUM and the final output is fp32. The
softmax denominator path (reciprocal) stays fp32.

Per-core device program (layouts chosen so no big intermediate needs a
transpose; only x is transposed once, on PE against a bf16 identity):
  x^T[k,s]   = PE transpose of x (cast to bf16 during the input DMA)
  Q^T/K^T    = Wq/Wk_cols^T @ x^T     -> [256, 2048] bf16, head-major
  V          = x @ Wv_cols            -> [2048, 256] natural, stored per
               kpos-tile with a ones column appended per head ([128, 4*65])
  scores^T   = K_h^T.T @ Q_h^T        -> [kpos 128, q 512] fp32 PSUM; the
               two heads of a 128-partition pair run row-packed (K=64,
               tile_position (0,0)/(64,0)) into two PSUM banks
  P^T        = exp(SCALE*scores^T + negmask[kpos])  (mask rides the ACT
               per-partition bias; scale is the free ACT affine) -> bf16
  ctx~^T     = [V_h | 1]^T @ P^T accumulated over kpos in PSUM -> [65, q];
               row 64 is the softmax denominator for free
  ctx^T     /= denom (fp32 reciprocal + PE broadcast via selector matmul)
  out^T     += Wo_rows^T @ ctx^T + bo_eff       (row-parallel partial, fp32)

bv is folded into bo_eff on the host (bv @ Wo_rows), bo added on one core
per batch group.
"""

import sys
import types

import ml_dtypes
import numpy as np

import concourse.bass as bass
import concourse.tile as tile
from concourse import bacc, mybir
from concourse.bass_utils import run_bass_kernel_spmd


def _install_ntff_hook_shim():
    """The agent image's antenv lacks axon_hooks, so trace=True dies on
    import. Recreate the module with the boot script's ctypes-based hook
    so NTFF profiling works."""
    if "antenv.axon_hooks" in sys.modules:
        return
    mod = types.ModuleType("antenv.axon_hooks")
    mod._hook = None

    def set_axon_ntff_profile_hook(h):
        mod._hook = h

    def get_axon_ntff_profile_hook():
        return mod._hook

    mod.set_axon_ntff_profile_hook = set_axon_ntff_profile_hook
    mod.get_axon_ntff_profile_hook = get_axon_ntff_profile_hook
    sys.modules["antenv.axon_hooks"] = mod
    try:
        from trn_agent_boot.trn_boot import _ntff_profile_via_ctypes

        mod._hook = _ntff_profile_via_ctypes("/opt/axon/libaxon_pjrt.so")
    except Exception:
        mod._hook = None


_install_ntff_hook_shim()

F32 = mybir.dt.float32
BF16 = mybir.dt.bfloat16

B = 2
S = 2048
HID = 1024
NH = 16  # total heads
DH = 64  # head dim
NCORES = 8
GROUPS = 4  # head groups (cores per batch)
NHC = 256  # hidden columns per core (4 heads * 64)
KT = 16  # kpos tiles of 128
SCALE = DH**-0.5

AF = mybir.ActivationFunctionType


def build_program(masked=False):
    nc = bacc.Bacc(
        "TRN2",
        target_bir_lowering=False,
        debug=False,
        enable_asserts=False,
        num_devices=NCORES,
    )

    # x is supplied pre-transposed (and bf16) by the host: x^T [HID, S]
    x_d = nc.dram_tensor("xt", [HID, S], BF16, kind="ExternalInput")
    # K and Q projection columns interleaved per pair so each pair's weights
    # arrive in ONE DMA: cols [wk0 | wq0 | wk1 | wq1], 128 each
    wkq_d = nc.dram_tensor("wkq", [HID, 2 * NHC], BF16, kind="ExternalInput")
    wv_d = nc.dram_tensor("wv", [HID, NHC], BF16, kind="ExternalInput")
    wo_d = nc.dram_tensor("wo", [NHC, HID], BF16, kind="ExternalInput")
    # small per-core constants batched into one DMA: bq | bk | negmask
    sm_d = nc.dram_tensor("small", [128, 4 + KT], F32, kind="ExternalInput")
    bo_d = nc.dram_tensor("bo", [128, 8], F32, kind="ExternalInput")
    sel_d = nc.dram_tensor("sel", [128, 2, 128], BF16, kind="ExternalInput")
    # bf16 partials: the host sums 4 per batch in fp32
    out_d = nc.dram_tensor("outT", [HID, S], BF16, kind="ExternalOutput")
    # extra partial for the last q-chunk: its pt0 half is projected
    # mid-stream (its denominators are ready after group 3), so the
    # post-stream tail only runs the pt1 half. Host adds this partial.
    out2_d = nc.dram_tensor("outU0", [HID, 512], BF16, kind="ExternalOutput")

    F32R = mybir.dt.float32r

    with tile.TileContext(nc) as tc:
        with tc.tile_pool(name="persist", bufs=1) as persist:
            small = persist.tile([128, 4 + KT], F32, tag="small")
            bq_sb = small[:, 0:2]
            bk_sb = small[:, 2:4]
            negm = small[:, 4 : 4 + KT]
            wo_sb = persist.tile([128, 2, HID], BF16, tag="wo")
            bo_sb = persist.tile([128, 8], F32, tag="bo")

            qt = persist.tile([128, 2, S], BF16, tag="qt")
            kt_sb = persist.tile([128, 2, S], BF16, tag="kt")
            vall = persist.tile([128, KT, 4 * 65], BF16, tag="vall")
            ctx_sb = persist.tile([128, 2, S], BF16, tag="ctx")
            # softmax denominators: head h on partition 32*h (engine APs must
            # start on a 32-partition boundary); unused partitions primed 1.0
            den = persist.tile([128, S], F32, tag="den")
            rec = persist.tile([128, S], F32, tag="rec")
            scr = persist.tile([128, S], F32, tag="scr")
            # small first slice on gpsimd so the PE prewarm isn't blocked
            # behind a 1.8us full-den DVE memset
            nc.gpsimd.memset(den[:, 0:128], 1.0)
            nc.gpsimd.memset(den[:, 128:], 1.0)
            # preload the ACT exp table set while input DMAs are in flight
            nc.scalar.activation(scr[:, 0:1], den[:, 0:1], AF.Exp)
            sel = persist.tile([128, 2, 128], BF16, tag="sel")
            rec_bf = persist.tile([128, S], BF16, tag="rec_bf")
            # the pt0-norm of the last chunk contracts sel over all 128
            # partitions; rows 64-127 of rec_bf are otherwise unwritten at
            # that point and must be finite
            nc.gpsimd.memset(rec_bf[64:128, 1536:2048], 1.0)

            # HAM prewarm: ~5us of dummy matmuls (on den, just memset)
            # so the free-running PE clock gate opens during the input-DMA
            # wait and the first real projections run at 2.4 GHz. Enough of
            # them that the PE stays warm until the first x/weight DMAs
            # land (~12.5us).
            warm_cm = tc.tile_pool(name="warm", bufs=1, space="PSUM")
            warm = warm_cm.__enter__()
            wps = warm.tile([128, 2, 128], F32, tag="wps")
            for wi in range(8):
                nc.tensor.matmul(
                    wps[:, wi % 2, :],
                    lhsT=den[:, 0:128],
                    rhs=den[:, 0:128],
                    start=True,
                    stop=True,
                )
            warm_cm.__exit__(None, None, None)

            out_re = out_d[:].rearrange("(a p) s -> p a s", p=128)
            out2_re = out2_d[:].rearrange("(a p) s -> p a s", p=128)

            # The whole kernel is emitted as one software-pipelined stream:
            # global attention iteration i = 0..127 (group g = i//16 =
            # (pair, q-chunk), t = i%16). exp(i) leads; scores run 2 ahead,
            # ctx LAG behind (so V production and ctx never block the exp
            # stream); x/K/Q/V production blocks are interleaved at fixed
            # stream positions. PSUM: sps 2x2 + cps 2x1 + ph1/ps_x 2 = 8.
            LAG = 10

            with tc.tile_pool(name="expp", bufs=28) as expp, \
                 tc.tile_pool(name="bcp", bufs=2) as bcp, \
                 tc.tile_pool(name="outp", bufs=2) as outp, \
                 tc.tile_pool(name="win", bufs=1) as win_p, \
                 tc.tile_pool(name="xtp", bufs=1) as xtp:

                ps_s_cm = tc.tile_pool(name="ps_s", bufs=2, space="PSUM")
                ps_s = ps_s_cm.__enter__()
                ps_c_cm = tc.tile_pool(name="ps_c", bufs=1, space="PSUM")
                ps_c = ps_c_cm.__enter__()
                ph1_cm = tc.tile_pool(name="ph1", bufs=2, space="PSUM")
                ph1 = ph1_cm.__enter__()

                kq_sb = win_p.tile([128, 8, 2 * NHC], BF16, tag="wkq")
                wv_sb = win_p.tile([128, 8, NHC], BF16, tag="wv")

                def kq_dma(pt, eng=None, part=None):
                    # one DMA per pair for both K and Q weights; pair 0 rides
                    # the scalar HWDGE ring, parallel with xt on the sync
                    # ring, split K-cols-first so the K projection (which
                    # leads the stream) can start half a transfer earlier
                    lo = pt * 256 + (128 if part == 1 else 0)
                    hi = pt * 256 + (128 if part == 0 else 256)
                    (eng or nc.sync).dma_start(
                        out=kq_sb[:, :, lo:hi],
                        in_=wkq_d[:].rearrange("(a p) n -> p a n", p=128)[
                            :, :, lo:hi
                        ],
                    )
                xT = xtp.tile([128, 8, S], BF16, tag="xT")

                # ---------- production helpers ----------
                xt_re = x_d[:].rearrange("(a p) s -> p a s", p=128)

                def xt_dma(c2, kh=None, width=512, eng=None, kr=None):
                    s0 = c2 * 512
                    if kr is not None:
                        k0, k1 = kr
                    else:
                        k0, k1 = (0, 8) if kh is None else (4 * kh, 4 * kh + 4)
                    (eng or nc.sync).dma_start(
                        out=xT[:, k0:k1, s0 : s0 + width],
                        in_=xt_re[:, k0:k1, s0 : s0 + width],
                    )

                pp_half = {}

                def proj_kq(off, bsb, dst, pt, j4, half=None, kr=None):
                    """One K/Q projection chunk (weight cols at `off` in
                    kq_sb); half=0/1 emits 4 of the 8 accumulating matmuls so
                    a chunk can straddle two stream slots without a long PE
                    burst blocking the exp stream. kr=(lo,hi) emits an
                    arbitrary kj range (used at startup to chase the x
                    DMA pieces)."""
                    key = (off, j4)
                    if kr is not None:
                        lo, hi = kr
                    elif half == 1:
                        lo, hi = 4, 8
                    elif half == 0:
                        lo, hi = 0, 4
                    else:
                        lo, hi = 0, 8
                    if lo == 0:
                        pp = ph1.tile([128, 512], F32, tag="ph1", name="pp")
                        if hi < 8:
                            pp_half[key] = pp
                    else:
                        pp = pp_half[key] if hi < 8 else pp_half.pop(key)
                    for kj in range(lo, hi):
                        nc.tensor.matmul(
                            pp,
                            lhsT=kq_sb[:, kj, off : off + 128],
                            rhs=xT[:, kj, j4 * 512 : (j4 + 1) * 512],
                            start=(kj == 0),
                            stop=(kj == 7),
                        )
                    if hi == 8:
                        nc.vector.tensor_scalar_add(
                            dst[:, pt, j4 * 512 : (j4 + 1) * 512],
                            pp,
                            bsb[:, pt : pt + 1],
                        )

                def proj_v(mt):
                    pv = ph1.tile([128, NHC], F32, tag="ph1", name="pv")
                    for kj in range(8):
                        nc.tensor.matmul(
                            pv,
                            lhsT=xT[:, kj, mt * 128 : (mt + 1) * 128],
                            rhs=wv_sb[:, kj, :],
                            start=(kj == 0),
                            stop=(kj == 7),
                        )
                    v_slot = vall[:, mt, :].rearrange("p (h e) -> p h e", h=4)
                    nc.vector.tensor_copy(
                        v_slot[:, :, 0:64], pv.rearrange("p (h d) -> p h d", h=4)
                    )
                    nc.gpsimd.memset(v_slot[:, :, 64:65], 1.0)

                # production schedule: stream position -> emitters.
                # c2 blocks feed group-0 scores just in time; V[t] must land
                # before ctx(t) at stream t+LAG; Q0[j]/K1/Q1 feed later groups.
                def late_dmas():
                    nc.sync.dma_start(
                        out=wo_sb, in_=wo_d[:].rearrange("(a p) n -> p a n", p=128)
                    )
                    nc.sync.dma_start(out=sel, in_=sel_d[:])
                    nc.sync.dma_start(out=bo_sb, in_=bo_d[:])

                def wv_dma():
                    nc.sync.dma_start(
                        out=wv_sb, in_=wv_d[:].rearrange("(a p) n -> p a n", p=128)
                    )

                def small_dma():
                    # tiny; rides the gpsimd SWDGE queue so the two HWDGE
                    # rings stay dedicated to x / weights during startup
                    nc.gpsimd.dma_start(out=small, in_=sm_d[:])

                def pk(pt, j4, half=None, kr=None):
                    proj_kq(pt * 256, bk_sb, kt_sb, pt, j4, half, kr)

                def pq(pt, j4, half=None, kr=None):
                    proj_kq(pt * 256 + 128, bq_sb, qt, pt, j4, half, kr)


                prod = {
                    # x arrives in 2-kj pieces and the first K/Q chunks are
                    # emitted in matching kj quarters, so the PE starts on
                    # the first 256KB piece instead of waiting for the full
                    # 512KB half. K weights land before Q so K leads.
                    -1: [lambda: kq_dma(0, nc.scalar, part=0),
                         lambda: kq_dma(0, nc.scalar, part=1),
                         lambda: xt_dma(0, kr=(0, 2)),
                         lambda: xt_dma(0, kr=(2, 4)),
                         lambda: xt_dma(0, kr=(4, 6)),
                         lambda: xt_dma(0, kr=(6, 8)),
                         small_dma,
                         lambda: pk(0, 0, kr=(0, 2)),
                         lambda: pk(0, 0, kr=(2, 4)),
                         lambda: pq(0, 0, kr=(0, 2)),
                         lambda: pq(0, 0, kr=(2, 4)),
                         lambda: pk(0, 0, kr=(4, 6)),
                         lambda: pq(0, 0, kr=(4, 6)),
                         lambda: pk(0, 0, kr=(6, 8)),
                         lambda: pq(0, 0, kr=(6, 8))],
                    0: [lambda: xt_dma(1)],
                    1: [wv_dma, lambda: pk(0, 1)],
                    3: [lambda: xt_dma(2)],
                    4: [lambda: pk(0, 2)],
                    5: [lambda: xt_dma(3)],
                    6: [lambda: pk(0, 3)],
                    8: [lambda: proj_v(0)],
                    9: [lambda: proj_v(1)],
                    10: [lambda: pq(0, 1)],
                    11: [lambda: proj_v(2)],
                    12: [lambda: proj_v(3)],
                    13: [lambda: proj_v(4)],
                    14: [lambda: proj_v(5)],
                    15: [lambda: proj_v(6)],
                    16: [lambda: proj_v(7)],
                    17: [lambda: proj_v(8)],
                    18: [lambda: proj_v(9), lambda: kq_dma(1)],
                    19: [lambda: proj_v(10)],
                    20: [lambda: proj_v(11)],
                    21: [lambda: proj_v(12)],
                    22: [lambda: proj_v(13)],
                    23: [lambda: proj_v(14)],
                    24: [lambda: proj_v(15)],
                    26: [late_dmas],
                    27: [lambda: pq(0, 2, 0)],
                    29: [lambda: pq(0, 2, 1)],
                    33: [lambda: pq(0, 3, 0)],
                    35: [lambda: pq(0, 3, 1)],
                    36: [lambda: pk(1, 0, 0)],
                    38: [lambda: pk(1, 0, 1)],
                    39: [lambda: pk(1, 1, 0)],
                    41: [lambda: pk(1, 1, 1)],
                    42: [lambda: pk(1, 2, 0)],
                    44: [lambda: pk(1, 2, 1)],
                    45: [lambda: pk(1, 3, 0)],
                    47: [lambda: pk(1, 3, 1)],
                    48: [lambda: pq(1, 0, 0)],
                    50: [lambda: pq(1, 0, 1)],
                    51: [lambda: pq(1, 1, 0)],
                    53: [lambda: pq(1, 1, 1)],
                    54: [lambda: pq(1, 2, 0)],
                    56: [lambda: pq(1, 2, 1)],
                    57: [lambda: pq(1, 3, 0)],
                    58: [lambda: pq(1, 3, 1)],
                }

                # ---------- attention stream state ----------
                # groups of (pt, q0, width): the final 512-chunk is split
                # into two 256-wide half-groups so the post-stream serial
                # tail (normalize + out-proj of the very last data) is half
                # as long, and the first half's tail overlaps the stream.
                groups_tbl = [
                    (0, 0, 512), (0, 512, 512), (0, 1024, 512), (0, 1536, 512),
                    (1, 0, 512), (1, 512, 512), (1, 1024, 512), (1, 1536, 512),
                ]
                N_IT = 16 * len(groups_tbl)
                LAST_G = len(groups_tbl) - 1
                cps_by_group = {}
                eps = {}
                sps_q = {}
                ps_x = None  # opened after ph1 closes (bank handoff)

                def scores(k):
                    g, t = k // 16, k % 16
                    pt, q0, W = groups_tbl[g]
                    # tiles stay full-width (one whole PSUM bank per head) so
                    # accumulation zero-regions are never shared across groups
                    sps = ps_s.tile([128, 2, 512], F32, tag="s", name="sps")
                    for hh in range(2):
                        nc.tensor.matmul(
                            sps[:, hh, 0:W],
                            lhsT=kt_sb[
                                hh * 64 : (hh + 1) * 64, pt, t * 128 : (t + 1) * 128
                            ],
                            rhs=qt[hh * 64 : (hh + 1) * 64, pt, q0 : q0 + W],
                            start=True,
                            stop=True,
                            tile_position=(hh * 64, 0),
                        )
                    sps_q[k] = sps

                def exp_step(k):
                    g, t = k // 16, k % 16
                    W = groups_tbl[g][2]
                    ep = expp.tile([128, 2, W], BF16, tag="e", name="ep")
                    # the additive mask is identically zero for all-ones
                    # attention_mask (the spec'd fill); skip the per-partition
                    # bias AP read in that case
                    bias = negm[:, t : t + 1] if masked else 0.0
                    nc.scalar.activation(
                        ep,
                        sps_q.pop(k)[:, :, 0:W],
                        AF.Exp,
                        bias=bias,
                        scale=float(SCALE),
                    )
                    eps[k] = ep

                def ctx_step(k):
                    g, t = k // 16, k % 16
                    pt, q0, W = groups_tbl[g]
                    if t == 0:
                        cps_by_group[g] = [
                            ps_c.tile([65, 512], F32, tag=f"c{h}", name=f"cps{h}")
                            for h in range(2)
                        ]
                    cps = cps_by_group[g]
                    ep = eps.pop(k)
                    for hh in range(2):
                        c0 = (2 * pt + hh) * 65
                        nc.tensor.matmul(
                            cps[hh][:, 0:W],
                            lhsT=vall[:, t, c0 : c0 + 65],
                            rhs=ep[:, hh, :],
                            start=(t == 0),
                            stop=(t == KT - 1),
                        )
                    if t == KT - 1:
                        finish_group(g)

                def finish_group(g):
                    pt, q0, W = groups_tbl[g]
                    cps = cps_by_group.pop(g)
                    if g == LAST_G:
                        # the whole post-stream chain hangs off these copies:
                        # emit them in 256-wide halves so the first half's
                        # reciprocal starts after ~1/4 of the copy work, with
                        # the two heads' den copies on different engines
                        for h2 in range(2):
                            lo, hi = h2 * (W // 2), (h2 + 1) * (W // 2)
                            for hh in range(2):
                                hd = 2 * pt + hh
                                den_dst = den[32 * hd : 32 * hd + 1, q0 + lo : q0 + hi]
                                if hh:
                                    nc.scalar.copy(den_dst, cps[hh][64:65, lo:hi])
                                else:
                                    nc.vector.tensor_copy(
                                        den_dst, cps[hh][64:65, lo:hi]
                                    )
                        for h2 in range(2):
                            lo, hi = h2 * (W // 2), (h2 + 1) * (W // 2)
                            for hh in range(2):
                                ctx_dst = ctx_sb[
                                    hh * 64 : (hh + 1) * 64, pt, q0 + lo : q0 + hi
                                ]
                                if hh:
                                    nc.scalar.copy(ctx_dst, cps[hh][0:64, lo:hi])
                                else:
                                    nc.vector.tensor_copy(
                                        ctx_dst, cps[hh][0:64, lo:hi]
                                    )
                        tail_chunk(q0, W, last=True)
                        return
                    for hh in range(2):
                        # hh=1 copies ride ACT so both heads' PSUM banks free
                        # in parallel — the next group's first ctx matmul
                        # (start=True) blocks the in-order PE queue until
                        # they do. Groups whose copies would interleave with
                        # the final exps (which gate the whole tail), or that
                        # land where the exp stream is ACT-saturated, keep
                        # the big ctx copy on DVE.
                        on_act = hh == 1 and g <= 3
                        ctx_dst = ctx_sb[hh * 64 : (hh + 1) * 64, pt, q0 : q0 + W]
                        h = 2 * pt + hh
                        den_dst = den[32 * h : 32 * h + 1, q0 : q0 + W]
                        # den first: it is tiny and gates the reciprocal,
                        # while the big ctx copy only gates the later muls.
                        # Late groups (g>=4) keep everything off ACT: the exp
                        # stream is ACT-saturated there and any ACT copy
                        # directly stalls the next scores pair via the sps
                        # WAR chain.
                        if on_act:
                            nc.scalar.copy(den_dst, cps[hh][64:65, 0:W])
                            nc.scalar.copy(ctx_dst, cps[hh][0:64, 0:W])
                        else:
                            nc.vector.tensor_copy(den_dst, cps[hh][64:65, 0:W])
                            nc.vector.tensor_copy(ctx_dst, cps[hh][0:64, 0:W])
                    if pt == 1:
                        tail_chunk(q0, W, last=False)
                    elif g == 3:
                        # the last chunk's pt0 half: normalize + project it
                        # mid-stream into its own partial (outU0) so the
                        # post-stream tail only handles the pt1 half
                        u0_tail(q0, W)

                pending = []
                final_pending = []
                tail_state = {}

                def tail_chunk(q0, W, last):
                    """Normalize q-range [q0, q0+W) + its output-projection
                    slice, split into small parts consumed one per stream
                    iteration so the PE burst never stalls the exp stream.

                    The LAST (half-width) chunk's chain runs post-stream from
                    a deep tail pool; everything else rides the stream."""
                    st = {}

                    if last:
                        # pt1-only: the pt0 half was normalized + projected
                        # mid-stream by u0_tail into its own partial. The
                        # post-stream chain runs in two pipelined 256-wide
                        # halves so the second half's reciprocal/normalize
                        # overlaps the first half's output projection.
                        HW2 = W // 2

                        def recip_h(h):
                            qh = q0 + h * HW2
                            # single-op approx recip (18 bits — the bf16
                            # broadcast path rounds to 8 anyway); full 128
                            # partitions: the custom DVE op is not trusted
                            # with a nonzero base partition
                            nc.vector.reciprocal_approx_fast(
                                out=rec[:, qh : qh + HW2],
                                in_=den[:, qh : qh + HW2],
                            )
                            nc.vector.tensor_copy(
                                rec_bf[:, qh : qh + HW2], rec[:, qh : qh + HW2]
                            )

                        def norm_h(h):
                            qh = q0 + h * HW2
                            bc = tail_state["pool"].tile(
                                [128, HW2], F32, tag="tl", name=f"bch{h}"
                            )
                            nc.tensor.matmul(
                                bc,
                                lhsT=sel[:, 1, :],
                                rhs=rec_bf[:, qh : qh + HW2],
                                start=True,
                                stop=True,
                            )
                            nc.vector.tensor_mul(
                                ctx_sb[:, 1, qh : qh + HW2],
                                ctx_sb[:, 1, qh : qh + HW2],
                                bc,
                            )

                        def out_h(h, mt2):
                            qh = q0 + h * HW2
                            key = f"ot{h}"
                            if mt2 == 0:
                                st[key] = outp.tile(
                                    [128, 8, HW2], BF16, tag="of", name=f"otf{h}"
                                )
                            ot = st[key]
                            tailp = tail_state["pool"]
                            for mt in (2 * mt2, 2 * mt2 + 1):
                                po = tailp.tile(
                                    [128, HW2], F32, tag="tl", name=f"pof{h}"
                                )
                                nc.tensor.matmul(
                                    po,
                                    lhsT=wo_sb[:, 1, mt * 128 : (mt + 1) * 128],
                                    rhs=ctx_sb[:, 1, qh : qh + HW2],
                                    start=True,
                                    stop=True,
                                )
                                if mt % 2 == 1:
                                    nc.scalar.add(
                                        ot[:, mt, :], po, bo_sb[:, mt : mt + 1]
                                    )
                                else:
                                    nc.vector.tensor_scalar_add(
                                        ot[:, mt, :], po, bo_sb[:, mt : mt + 1]
                                    )
                            # merged stores (4 mt rows each) on the idle sync
                            # ring: halves the serialized issuance train that
                            # otherwise dominates the post-compute tail, and
                            # keeps DMA issuances off the scalar queue where
                            # they would block the ACT adds
                            if mt2 in (1, 3):
                                m0 = 4 * (mt2 // 2)
                                nc.sync.dma_start(
                                    out=out_re[:, m0 : m0 + 4, qh : qh + HW2],
                                    in_=ot[:, m0 : m0 + 4, :],
                                )

                        final_pending.append(lambda: recip_h(0))
                        final_pending.append(lambda: norm_h(0))
                        final_pending.append(lambda: recip_h(1))
                        for mt2 in range(2):
                            final_pending.append(lambda mt2=mt2: out_h(0, mt2))
                        final_pending.append(lambda: norm_h(1))
                        for mt2 in range(2, 4):
                            final_pending.append(lambda mt2=mt2: out_h(0, mt2))
                        for mt2 in range(4):
                            final_pending.append(lambda mt2=mt2: out_h(1, mt2))
                        return

                    def p_recip():
                        nc.vector.reciprocal_approx_fast(
                            out=rec[:, q0 : q0 + W],
                            in_=den[:, q0 : q0 + W],
                        )
                        nc.vector.tensor_copy(
                            rec_bf[:, q0 : q0 + W], rec[:, q0 : q0 + W]
                        )

                    def p_norm(pt):
                        bc = ps_x.tile([128, W], F32, tag="tp", name="bc")
                        nc.tensor.matmul(
                            bc,
                            lhsT=sel[:, pt, :],
                            rhs=rec_bf[:, q0 : q0 + W],
                            start=True,
                            stop=True,
                        )
                        nc.vector.tensor_mul(
                            ctx_sb[:, pt, q0 : q0 + W],
                            ctx_sb[:, pt, q0 : q0 + W],
                            bc,
                        )

                    def p_out(mt2):
                        if mt2 == 0:
                            st["ot"] = outp.tile(
                                [128, 8, W], BF16, tag="o", name="ot"
                            )
                        ot = st["ot"]
                        for mt in (2 * mt2, 2 * mt2 + 1):
                            po = ps_x.tile([128, W], F32, tag="tp", name="po")
                            for pt in range(2):
                                nc.tensor.matmul(
                                    po,
                                    lhsT=wo_sb[:, pt, mt * 128 : (mt + 1) * 128],
                                    rhs=ctx_sb[:, pt, q0 : q0 + W],
                                    start=(pt == 0),
                                    stop=(pt == 1),
                                )
                                # (accumulates over pt)
                            nc.vector.tensor_scalar_add(
                                ot[:, mt, :], po, bo_sb[:, mt : mt + 1]
                            )
                        # store each finished pair immediately: spreads DMA and
                        # shrinks the post-stream tail to the last pair only
                        nc.sync.dma_start(
                            out=out_re[:, 2 * mt2 : 2 * mt2 + 2, q0 : q0 + W],
                            in_=ot[:, 2 * mt2 : 2 * mt2 + 2, :],
                        )
                        if mt2 == 3:
                            st.pop("ot")

                    pending.append(p_recip)
                    pending.append(lambda: p_norm(0))
                    pending.append(lambda: p_norm(1))
                    for mt2 in range(4):
                        pending.append(lambda mt2=mt2: p_out(mt2))

                def u0_tail(q0, W):
                    """Normalize + project the LAST chunk's pt0 half into the
                    outU0 partial while the stream is still running on pt1
                    groups (its denominators are complete after group 3)."""
                    st = {}

                    def u_recip():
                        nc.vector.reciprocal_approx_fast(
                            out=rec[0:64, q0 : q0 + W],
                            in_=den[0:64, q0 : q0 + W],
                        )
                        nc.vector.tensor_copy(
                            rec_bf[0:64, q0 : q0 + W], rec[0:64, q0 : q0 + W]
                        )

                    def u_norm():
                        bc = ps_x.tile([128, W], F32, tag="tp", name="ubc")
                        nc.tensor.matmul(
                            bc,
                            lhsT=sel[:, 0, :],
                            rhs=rec_bf[:, q0 : q0 + W],
                            start=True,
                            stop=True,
                        )
                        nc.vector.tensor_mul(
                            ctx_sb[:, 0, q0 : q0 + W],
                            ctx_sb[:, 0, q0 : q0 + W],
                            bc,
                        )

                    def u_out(mt2):
                        if mt2 == 0:
                            st["ot"] = outp.tile(
                                [128, 8, W], BF16, tag="o", name="otu"
                            )
                        ot = st["ot"]
                        for mt in (2 * mt2, 2 * mt2 + 1):
                            po = ps_x.tile([128, W], F32, tag="tp", name="upo")
                            nc.tensor.matmul(
                                po,
                                lhsT=wo_sb[:, 0, mt * 128 : (mt + 1) * 128],
                                rhs=ctx_sb[:, 0, q0 : q0 + W],
                                start=True,
                                stop=True,
                            )
                            # bias rides the pt1 partial; this one is pure copy
                            nc.vector.tensor_copy(ot[:, mt, :], po)
                        nc.sync.dma_start(
                            out=out2_re[:, 2 * mt2 : 2 * mt2 + 2, :],
                            in_=ot[:, 2 * mt2 : 2 * mt2 + 2, :],
                        )
                        if mt2 == 3:
                            st.pop("ot")

                    pending.append(u_recip)
                    pending.append(u_norm)
                    for mt2 in range(4):
                        pending.append(lambda mt2=mt2: u_out(mt2))

                # ---------- the stream ----------
                # graduated ctx lag: large early (the first half is PE-bound
                # on projections; ctx shifts into pt1's slack), catching up
                # two-per-iteration once the projections are done, then a
                # short lag so the epilogue stays small.
                for fn in prod[-1]:
                    fn()
                scores(0)
                scores(1)
                ctx_next = 0
                for i in range(N_IT):
                    exp_step(i)
                    if i + 2 < N_IT:
                        scores(i + 2)
                    for fn in prod.get(i, []):
                        fn()
                    if i == 62:
                        # last ph1 use emitted: release its 2 PSUM banks so
                        # the tail pool can take them over
                        ph1_cm.__exit__(None, None, None)
                        ps_x_cm = tc.tile_pool(name="ps_x", bufs=2, space="PSUM")
                        ps_x = ps_x_cm.__enter__()
                    want = 0
                    if i >= 26:
                        want = 1
                    if 64 <= i and i % 2 == 0 and ctx_next < i - 4:
                        want = 2
                    if i >= 104 and ctx_next < i - 2:
                        want = 2
                    if i >= N_IT - 10 and not pending and ctx_next < i:
                        want = 2
                    for j in range(want):
                        if ctx_next <= i:
                            # don't start a new group's ctx (start=True, which
                            # waits on the previous group's PSUM-evacuation
                            # copies) back-to-back with the old group's last
                            # step — defer one iteration so the copies finish
                            if (
                                j == 1
                                and ctx_next % KT == 0
                                and i < N_IT - 12
                            ):
                                break
                            ctx_step(ctx_next)
                            ctx_next += 1
                    if pending and (i % 2 == 1 or i >= 108):
                        pending.pop(0)()
                        if i >= 112 and pending:
                            pending.pop(0)()
                while ctx_next < N_IT:
                    ctx_step(ctx_next)
                    ctx_next += 1
                    if pending:
                        pending.pop(0)()
                while pending:
                    pending.pop(0)()
                # all score/ctx PSUM users emitted: hand their banks to a
                # deep tail pool so the final out-proj pipelines freely
                ps_x_cm.__exit__(None, None, None)
                ps_c_cm.__exit__(None, None, None)
                ps_s_cm.__exit__(None, None, None)
                tailp_cm = tc.tile_pool(name="tailp", bufs=8, space="PSUM")
                tail_state["pool"] = tailp_cm.__enter__()
                while final_pending:
                    final_pending.pop(0)()
                tailp_cm.__exit__(None, None, None)

    nc.compile()
    return nc

_PROGRAMS = {}


def _get_program(masked=False):
    if masked not in _PROGRAMS:
        _PROGRAMS[masked] = build_program(masked)
    return _PROGRAMS[masked]


def make_in_maps(inputs):
    hs = np.asarray(inputs["hidden_states"], dtype=np.float32)
    mask = np.asarray(inputs["attention_mask"], dtype=np.float32)
    Wq = np.asarray(inputs["Wq"], dtype=np.float32)
    bq = np.asarray(inputs["bq"], dtype=np.float32)
    Wk = np.asarray(inputs["Wk"], dtype=np.float32)
    bk = np.asarray(inputs["bk"], dtype=np.float32)
    Wv = np.asarray(inputs["Wv"], dtype=np.float32)
    bv = np.asarray(inputs["bv"], dtype=np.float32)
    Wo = np.asarray(inputs["Wo"], dtype=np.float32)
    bo = np.asarray(inputs["bo"], dtype=np.float32)

    # selector: sel[k, pt, m] = 1 iff k == 32*(2*pt + m//64) (same on all
    # cores; head h's reciprocal lives on partition 32*h)
    sel = np.zeros((128, 2, 128), np.float32)
    for pt in range(2):
        for hh in range(2):
            sel[32 * (2 * pt + hh), pt, hh * 64 : (hh + 1) * 64] = 1.0

    in_maps = []
    for c in range(NCORES):
        b = c // GROUPS
        g = c % GROUPS
        cols = slice(g * NHC, (g + 1) * NHC)
        negm = ((1.0 - mask[b]) * -10000.0).astype(np.float32)
        # bv folded through the output projection; bo added on one core/batch
        bo_eff = bv[cols] @ Wo[cols, :]
        if g == 0:
            bo_eff = bo_eff + bo
        bf = ml_dtypes.bfloat16
        wkc = Wk[:, cols]
        wqc = Wq[:, cols]
        wkq = np.concatenate(
            [wkc[:, 0:128], wqc[:, 0:128], wkc[:, 128:256], wqc[:, 128:256]],
            axis=1,
        )
        in_maps.append(
            {
                "xt": np.ascontiguousarray(hs[b].T.astype(bf)),
                "wkq": np.ascontiguousarray(wkq.astype(bf)),
                "wv": np.ascontiguousarray(Wv[:, cols].astype(bf)),
                "wo": np.ascontiguousarray(Wo[cols, :].astype(bf)),
                "small": np.ascontiguousarray(
                    np.concatenate(
                        [
                            bq[cols].reshape(2, 128).T,
                            bk[cols].reshape(2, 128).T,
                            negm.reshape(KT, 128).T,
                        ],
                        axis=1,
                    ).astype(np.float32)
                ),
                "bo": np.ascontiguousarray(
                    bo_eff.astype(np.float32).reshape(8, 128).T
                ),
                "sel": sel.astype(ml_dtypes.bfloat16),
            }
        )
    return in_maps


def gather_output(per_core_outs):
    out = np.empty((B, S, HID), dtype=np.float32)
    for b in range(B):
        acc = per_core_outs[b * GROUPS]["outT"].astype(np.float32)
        for g in range(1, GROUPS):
            acc = acc + per_core_outs[b * GROUPS + g]["outT"].astype(np.float32)
        # the last q-chunk's pt0 half lives in its own partial per core
        for g in range(GROUPS):
            acc[:, 1536:2048] += per_core_outs[b * GROUPS + g]["outU0"].astype(
                np.float32
            )
        out[b] = acc.T
    return out


def run(inputs, trace=False):
    mask = np.asarray(inputs["attention_mask"], dtype=np.float32)
    nc = _get_program(masked=bool((mask != 1.0).any()))
    in_maps = make_in_maps(inputs)
    res = run_bass_kernel_spmd(
        nc, in_maps, core_ids=list(range(NCORES)), trace=trace
    )
    return gather_output(res.results), res


def kernel(**inputs):
    out, _ = run(inputs, trace=False)
    return out



# revision 38
# speedup vs baseline: 1.0024x; 1.0024x over previous
"""Multi-head attention kernel for Trainium2, sharded over 8 NeuronCores.

Problem: B=2, S=2048, HIDDEN=1024, 16 heads, head_dim=64, fp32 in/out.

Sharding (data + tensor parallel per the hint): core c handles batch b=c//4
and head-group g=c%4 (4 heads = 256 hidden columns). QKV projections are
column-sharded, output projection row-sharded; each core returns a partial
out^T [1024, 2048] and the host sums the 4 partials per batch (the
row-parallel all-reduce) and transposes.

All matmul operands are bf16 (fp32 matmul runs at 1/4 PE rate: 2 half-speed
passes); accumulation is fp32 in PSUM and the final output is fp32. The
softmax denominator path (reciprocal) stays fp32.

Per-core device program (layouts chosen so no big intermediate needs a
transpose; only x is transposed once, on PE against a bf16 identity):
  x^T[k,s]   = PE transpose of x (cast to bf16 during the input DMA)
  Q^T/K^T    = Wq/Wk_cols^T @ x^T     -> [256, 2048] bf16, head-major
  V          = x @ Wv_cols            -> [2048, 256] natural, stored per
               kpos-tile with a ones column appended per head ([128, 4*65])
  scores^T   = K_h^T.T @ Q_h^T        -> [kpos 128, q 512] fp32 PSUM; the
               two heads of a 128-partition pair run row-packed (K=64,
               tile_position (0,0)/(64,0)) into two PSUM banks
  P^T        = exp(SCALE*scores^T + negmask[kpos])  (mask rides the ACT
               per-partition bias; scale is the free ACT affine) -> bf16
  ctx~^T     = [V_h | 1]^T @ P^T accumulated over kpos in PSUM -> [65, q];
               row 64 is the softmax denominator for free
  ctx^T     /= denom (fp32 reciprocal + PE broadcast via selector matmul)
  out^T     += Wo_rows^T @ ctx^T + bo_eff       (row-parallel partial, fp32)

bv is folded into bo_eff on the host (bv @ Wo_rows), bo added on one core
per batch group.
"""

import sys
import types

import ml_dtypes
import numpy as np

import concourse.bass as bass
import concourse.tile as tile
from concourse import bacc, mybir
from concourse.bass_utils import run_bass_kernel_spmd


def _install_ntff_hook_shim():
    """The agent image's antenv lacks axon_hooks, so trace=True dies on
    import. Recreate the module with the boot script's ctypes-based hook
    so NTFF profiling works."""
    if "antenv.axon_hooks" in sys.modules:
        return
    mod = types.ModuleType("antenv.axon_hooks")
    mod._hook = None

    def set_axon_ntff_profile_hook(h):
        mod._hook = h

    def get_axon_ntff_profile_hook():
        return mod._hook

    mod.set_axon_ntff_profile_hook = set_axon_ntff_profile_hook
    mod.get_axon_ntff_profile_hook = get_axon_ntff_profile_hook
    sys.modules["antenv.axon_hooks"] = mod
    try:
        from trn_agent_boot.trn_boot import _ntff_profile_via_ctypes

        mod._hook = _ntff_profile_via_ctypes("/opt/axon/libaxon_pjrt.so")
    except Exception:
        mod._hook = None


_install_ntff_hook_shim()

F32 = mybir.dt.float32
BF16 = mybir.dt.bfloat16

B = 2
S = 2048
HID = 1024
NH = 16  # total heads
DH = 64  # head dim
NCORES = 8
GROUPS = 4  # head groups (cores per batch)
NHC = 256  # hidden columns per core (4 heads * 64)
KT = 16  # kpos tiles of 128
SCALE = DH**-0.5

AF = mybir.ActivationFunctionType


def build_program(masked=False):
    nc = bacc.Bacc(
        "TRN2",
        target_bir_lowering=False,
        debug=False,
        enable_asserts=False,
        num_devices=NCORES,
    )

    # x is supplied pre-transposed (and bf16) by the host: x^T [HID, S]
    x_d = nc.dram_tensor("xt", [HID, S], BF16, kind="ExternalInput")
    # K and Q projection columns interleaved per pair so each pair's weights
    # arrive in ONE DMA: cols [wk0 | wq0 | wk1 | wq1], 128 each
    wkq_d = nc.dram_tensor("wkq", [HID, 2 * NHC], BF16, kind="ExternalInput")
    wv_d = nc.dram_tensor("wv", [HID, NHC], BF16, kind="ExternalInput")
    wo_d = nc.dram_tensor("wo", [NHC, HID], BF16, kind="ExternalInput")
    # small per-core constants batched into one DMA: bq | bk | negmask
    sm_d = nc.dram_tensor("small", [128, 4 + KT], F32, kind="ExternalInput")
    bo_d = nc.dram_tensor("bo", [128, 8], F32, kind="ExternalInput")
    sel_d = nc.dram_tensor("sel", [128, 2, 128], BF16, kind="ExternalInput")
    # bf16 partials: the host sums 4 per batch in fp32
    out_d = nc.dram_tensor("outT", [HID, S], BF16, kind="ExternalOutput")
    # extra partial for the last q-chunk: its pt0 half is projected
    # mid-stream (its denominators are ready after group 3), so the
    # post-stream tail only runs the pt1 half. Host adds this partial.
    out2_d = nc.dram_tensor("outU0", [HID, 512], BF16, kind="ExternalOutput")

    F32R = mybir.dt.float32r

    with tile.TileContext(nc) as tc:
        with tc.tile_pool(name="persist", bufs=1) as persist:
            small = persist.tile([128, 4 + KT], F32, tag="small")
            bq_sb = small[:, 0:2]
            bk_sb = small[:, 2:4]
            negm = small[:, 4 : 4 + KT]
            wo_sb = persist.tile([128, 2, HID], BF16, tag="wo")
            bo_sb = persist.tile([128, 8], F32, tag="bo")

            qt = persist.tile([128, 2, S], BF16, tag="qt")
            kt_sb = persist.tile([128, 2, S], BF16, tag="kt")
            vall = persist.tile([128, KT, 4 * 65], BF16, tag="vall")
            ctx_sb = persist.tile([128, 2, S], BF16, tag="ctx")
            # softmax denominators: head h on partition 32*h (engine APs must
            # start on a 32-partition boundary); unused partitions primed 1.0
            den = persist.tile([128, S], F32, tag="den")
            rec = persist.tile([128, S], F32, tag="rec")
            scr = persist.tile([128, S], F32, tag="scr")
            # small first slice on gpsimd so the PE prewarm isn't blocked
            # behind a 1.8us full-den DVE memset
            nc.gpsimd.memset(den[:, 0:128], 1.0)
            nc.gpsimd.memset(den[:, 128:], 1.0)
            # preload the ACT exp table set while input DMAs are in flight
            nc.scalar.activation(scr[:, 0:1], den[:, 0:1], AF.Exp)
            sel = persist.tile([128, 2, 128], BF16, tag="sel")
            rec_bf = persist.tile([128, S], BF16, tag="rec_bf")
            # the pt0-norm of the last chunk contracts sel over all 128
            # partitions; rows 64-127 of rec_bf are otherwise unwritten at
            # that point and must be finite
            nc.gpsimd.memset(rec_bf[64:128, 1536:2048], 1.0)

            # HAM prewarm: ~5us of dummy matmuls (on den, just memset)
            # so the free-running PE clock gate opens during the input-DMA
            # wait and the first real projections run at 2.4 GHz. Enough of
            # them that the PE stays warm until the first x/weight DMAs
            # land (~12.5us).
            warm_cm = tc.tile_pool(name="warm", bufs=1, space="PSUM")
            warm = warm_cm.__enter__()
            wps = warm.tile([128, 2, 128], F32, tag="wps")
            for wi in range(8):
                nc.tensor.matmul(
                    wps[:, wi % 2, :],
                    lhsT=den[:, 0:128],
                    rhs=den[:, 0:128],
                    start=True,
                    stop=True,
                )
            warm_cm.__exit__(None, None, None)

            out_re = out_d[:].rearrange("(a p) s -> p a s", p=128)
            out2_re = out2_d[:].rearrange("(a p) s -> p a s", p=128)

            # The whole kernel is emitted as one software-pipelined stream:
            # global attention iteration i = 0..127 (group g = i//16 =
            # (pair, q-chunk), t = i%16). exp(i) leads; scores run 2 ahead,
            # ctx LAG behind (so V production and ctx never block the exp
            # stream); x/K/Q/V production blocks are interleaved at fixed
            # stream positions. PSUM: sps 2x2 + cps 2x1 + ph1/ps_x 2 = 8.
            LAG = 10

            with tc.tile_pool(name="expp", bufs=28) as expp, \
                 tc.tile_pool(name="bcp", bufs=2) as bcp, \
                 tc.tile_pool(name="outp", bufs=2) as outp, \
                 tc.tile_pool(name="win", bufs=1) as win_p, \
                 tc.tile_pool(name="xtp", bufs=1) as xtp:

                # ps_c first: it must outlive ps_s/ps_x (LIFO pool stack) —
                # the last group's PSUM evacuation runs inside the final tail
                ps_c_cm = tc.tile_pool(name="ps_c", bufs=1, space="PSUM")
                ps_c = ps_c_cm.__enter__()
                ps_s_cm = tc.tile_pool(name="ps_s", bufs=2, space="PSUM")
                ps_s = ps_s_cm.__enter__()
                ph1_cm = tc.tile_pool(name="ph1", bufs=2, space="PSUM")
                ph1 = ph1_cm.__enter__()

                kq_sb = win_p.tile([128, 8, 2 * NHC], BF16, tag="wkq")
                wv_sb = win_p.tile([128, 8, NHC], BF16, tag="wv")

                def kq_dma(pt, eng=None, part=None):
                    # one DMA per pair for both K and Q weights; pair 0 rides
                    # the scalar HWDGE ring, parallel with xt on the sync
                    # ring, split K-cols-first so the K projection (which
                    # leads the stream) can start half a transfer earlier
                    lo = pt * 256 + (128 if part == 1 else 0)
                    hi = pt * 256 + (128 if part == 0 else 256)
                    (eng or nc.sync).dma_start(
                        out=kq_sb[:, :, lo:hi],
                        in_=wkq_d[:].rearrange("(a p) n -> p a n", p=128)[
                            :, :, lo:hi
                        ],
                    )
                xT = xtp.tile([128, 8, S], BF16, tag="xT")

                # ---------- production helpers ----------
                xt_re = x_d[:].rearrange("(a p) s -> p a s", p=128)

                def xt_dma(c2, kh=None, width=512, eng=None, kr=None):
                    s0 = c2 * 512
                    if kr is not None:
                        k0, k1 = kr
                    else:
                        k0, k1 = (0, 8) if kh is None else (4 * kh, 4 * kh + 4)
                    (eng or nc.sync).dma_start(
                        out=xT[:, k0:k1, s0 : s0 + width],
                        in_=xt_re[:, k0:k1, s0 : s0 + width],
                    )

                pp_half = {}

                def proj_kq(off, bsb, dst, pt, j4, half=None, kr=None):
                    """One K/Q projection chunk (weight cols at `off` in
                    kq_sb); half=0/1 emits 4 of the 8 accumulating matmuls so
                    a chunk can straddle two stream slots without a long PE
                    burst blocking the exp stream. kr=(lo,hi) emits an
                    arbitrary kj range (used at startup to chase the x
                    DMA pieces)."""
                    key = (off, j4)
                    if kr is not None:
                        lo, hi = kr
                    elif half == 1:
                        lo, hi = 4, 8
                    elif half == 0:
                        lo, hi = 0, 4
                    else:
                        lo, hi = 0, 8
                    if lo == 0:
                        pp = ph1.tile([128, 512], F32, tag="ph1", name="pp")
                        if hi < 8:
                            pp_half[key] = pp
                    else:
                        pp = pp_half[key] if hi < 8 else pp_half.pop(key)
                    for kj in range(lo, hi):
                        nc.tensor.matmul(
                            pp,
                            lhsT=kq_sb[:, kj, off : off + 128],
                            rhs=xT[:, kj, j4 * 512 : (j4 + 1) * 512],
                            start=(kj == 0),
                            stop=(kj == 7),
                        )
                    if hi == 8:
                        nc.vector.tensor_scalar_add(
                            dst[:, pt, j4 * 512 : (j4 + 1) * 512],
                            pp,
                            bsb[:, pt : pt + 1],
                        )

                def proj_v(mt):
                    pv = ph1.tile([128, NHC], F32, tag="ph1", name="pv")
                    for kj in range(8):
                        nc.tensor.matmul(
                            pv,
                            lhsT=xT[:, kj, mt * 128 : (mt + 1) * 128],
                            rhs=wv_sb[:, kj, :],
                            start=(kj == 0),
                            stop=(kj == 7),
                        )
                    v_slot = vall[:, mt, :].rearrange("p (h e) -> p h e", h=4)
                    nc.vector.tensor_copy(
                        v_slot[:, :, 0:64], pv.rearrange("p (h d) -> p h d", h=4)
                    )
                    nc.gpsimd.memset(v_slot[:, :, 64:65], 1.0)

                # production schedule: stream position -> emitters.
                # c2 blocks feed group-0 scores just in time; V[t] must land
                # before ctx(t) at stream t+LAG; Q0[j]/K1/Q1 feed later groups.
                def late_dmas():
                    nc.sync.dma_start(
                        out=wo_sb, in_=wo_d[:].rearrange("(a p) n -> p a n", p=128)
                    )
                    nc.sync.dma_start(out=sel, in_=sel_d[:])
                    nc.sync.dma_start(out=bo_sb, in_=bo_d[:])

                def wv_dma():
                    nc.sync.dma_start(
                        out=wv_sb, in_=wv_d[:].rearrange("(a p) n -> p a n", p=128)
                    )

                def small_dma():
                    # tiny; rides the gpsimd SWDGE queue so the two HWDGE
                    # rings stay dedicated to x / weights during startup
                    nc.gpsimd.dma_start(out=small, in_=sm_d[:])

                def pk(pt, j4, half=None, kr=None):
                    proj_kq(pt * 256, bk_sb, kt_sb, pt, j4, half, kr)

                def pq(pt, j4, half=None, kr=None):
                    proj_kq(pt * 256 + 128, bq_sb, qt, pt, j4, half, kr)


                prod = {
                    # x arrives in 2-kj pieces and the first K/Q chunks are
                    # emitted in matching kj quarters, so the PE starts on
                    # the first 256KB piece instead of waiting for the full
                    # 512KB half. K weights land before Q so K leads.
                    -1: [lambda: kq_dma(0, nc.scalar, part=0),
                         lambda: kq_dma(0, nc.scalar, part=1),
                         lambda: xt_dma(0, kr=(0, 2)),
                         lambda: xt_dma(0, kr=(2, 4)),
                         lambda: xt_dma(0, kr=(4, 6)),
                         lambda: xt_dma(0, kr=(6, 8)),
                         small_dma,
                         lambda: pk(0, 0, kr=(0, 2)),
                         lambda: pk(0, 0, kr=(2, 4)),
                         lambda: pq(0, 0, kr=(0, 2)),
                         lambda: pq(0, 0, kr=(2, 4)),
                         lambda: pk(0, 0, kr=(4, 6)),
                         lambda: pq(0, 0, kr=(4, 6)),
                         lambda: pk(0, 0, kr=(6, 8)),
                         lambda: pq(0, 0, kr=(6, 8))],
                    0: [lambda: xt_dma(1)],
                    1: [wv_dma, lambda: pk(0, 1)],
                    3: [lambda: xt_dma(2)],
                    4: [lambda: pk(0, 2)],
                    5: [lambda: xt_dma(3)],
                    6: [lambda: pk(0, 3)],
                    8: [lambda: proj_v(0)],
                    9: [lambda: proj_v(1)],
                    10: [lambda: pq(0, 1)],
                    11: [lambda: proj_v(2)],
                    12: [lambda: proj_v(3)],
                    13: [lambda: proj_v(4)],
                    14: [lambda: proj_v(5)],
                    15: [lambda: proj_v(6)],
                    16: [lambda: proj_v(7)],
                    17: [lambda: proj_v(8)],
                    18: [lambda: proj_v(9), lambda: kq_dma(1)],
                    19: [lambda: proj_v(10)],
                    20: [lambda: proj_v(11)],
                    21: [lambda: proj_v(12)],
                    22: [lambda: proj_v(13)],
                    23: [lambda: proj_v(14)],
                    24: [lambda: proj_v(15)],
                    26: [late_dmas],
                    27: [lambda: pq(0, 2, 0)],
                    29: [lambda: pq(0, 2, 1)],
                    33: [lambda: pq(0, 3, 0)],
                    35: [lambda: pq(0, 3, 1)],
                    36: [lambda: pk(1, 0, 0)],
                    38: [lambda: pk(1, 0, 1)],
                    39: [lambda: pk(1, 1, 0)],
                    41: [lambda: pk(1, 1, 1)],
                    42: [lambda: pk(1, 2, 0)],
                    44: [lambda: pk(1, 2, 1)],
                    45: [lambda: pk(1, 3, 0)],
                    47: [lambda: pk(1, 3, 1)],
                    48: [lambda: pq(1, 0, 0)],
                    50: [lambda: pq(1, 0, 1)],
                    51: [lambda: pq(1, 1, 0)],
                    53: [lambda: pq(1, 1, 1)],
                    54: [lambda: pq(1, 2, 0)],
                    56: [lambda: pq(1, 2, 1)],
                    57: [lambda: pq(1, 3, 0)],
                    58: [lambda: pq(1, 3, 1)],
                }

                # ---------- attention stream state ----------
                # groups of (pt, q0, width): the final 512-chunk is split
                # into two 256-wide half-groups so the post-stream serial
                # tail (normalize + out-proj of the very last data) is half
                # as long, and the first half's tail overlaps the stream.
                groups_tbl = [
                    (0, 0, 512), (0, 512, 512), (0, 1024, 512), (0, 1536, 512),
                    (1, 0, 512), (1, 512, 512), (1, 1024, 512), (1, 1536, 512),
                ]
                N_IT = 16 * len(groups_tbl)
                LAST_G = len(groups_tbl) - 1
                cps_by_group = {}
                eps = {}
                sps_q = {}
                ps_x = None  # opened after ph1 closes (bank handoff)

                def scores(k):
                    g, t = k // 16, k % 16
                    pt, q0, W = groups_tbl[g]
                    # tiles stay full-width (one whole PSUM bank per head) so
                    # accumulation zero-regions are never shared across groups
                    sps = ps_s.tile([128, 2, 512], F32, tag="s", name="sps")
                    for hh in range(2):
                        nc.tensor.matmul(
                            sps[:, hh, 0:W],
                            lhsT=kt_sb[
                                hh * 64 : (hh + 1) * 64, pt, t * 128 : (t + 1) * 128
                            ],
                            rhs=qt[hh * 64 : (hh + 1) * 64, pt, q0 : q0 + W],
                            start=True,
                            stop=True,
                            tile_position=(hh * 64, 0),
                        )
                    sps_q[k] = sps

                def exp_step(k):
                    g, t = k // 16, k % 16
                    W = groups_tbl[g][2]
                    ep = expp.tile([128, 2, W], BF16, tag="e", name="ep")
                    # the additive mask is identically zero for all-ones
                    # attention_mask (the spec'd fill); skip the per-partition
                    # bias AP read in that case
                    bias = negm[:, t : t + 1] if masked else 0.0
                    nc.scalar.activation(
                        ep,
                        sps_q.pop(k)[:, :, 0:W],
                        AF.Exp,
                        bias=bias,
                        scale=float(SCALE),
                    )
                    eps[k] = ep

                def ctx_step(k):
                    g, t = k // 16, k % 16
                    pt, q0, W = groups_tbl[g]
                    if t == 0:
                        cps_by_group[g] = [
                            ps_c.tile([65, 512], F32, tag=f"c{h}", name=f"cps{h}")
                            for h in range(2)
                        ]
                    cps = cps_by_group[g]
                    ep = eps.pop(k)
                    for hh in range(2):
                        c0 = (2 * pt + hh) * 65
                        nc.tensor.matmul(
                            cps[hh][:, 0:W],
                            lhsT=vall[:, t, c0 : c0 + 65],
                            rhs=ep[:, hh, :],
                            start=(t == 0),
                            stop=(t == KT - 1),
                        )
                    if t == KT - 1:
                        finish_group(g)

                def finish_group(g):
                    pt, q0, W = groups_tbl[g]
                    cps = cps_by_group.pop(g)
                    if g == LAST_G:
                        # the whole post-stream chain hangs off the copies:
                        # tail_chunk interleaves them in 256-wide halves with
                        # the reciprocal/normalize chain so the first half's
                        # reciprocal starts after ~1/4 of the copy work
                        tail_chunk(q0, W, last=True, cps=cps, pt=pt)
                        return
                    for hh in range(2):
                        # hh=1 copies ride ACT so both heads' PSUM banks free
                        # in parallel — the next group's first ctx matmul
                        # (start=True) blocks the in-order PE queue until
                        # they do. Groups whose copies would interleave with
                        # the final exps (which gate the whole tail), or that
                        # land where the exp stream is ACT-saturated, keep
                        # the big ctx copy on DVE.
                        on_act = hh == 1 and g <= 3
                        ctx_dst = ctx_sb[hh * 64 : (hh + 1) * 64, pt, q0 : q0 + W]
                        h = 2 * pt + hh
                        den_dst = den[32 * h : 32 * h + 1, q0 : q0 + W]
                        # den first: it is tiny and gates the reciprocal,
                        # while the big ctx copy only gates the later muls.
                        # For late groups only the tiny den copy rides ACT
                        # (the exp stream is ACT-saturated there) so the cps
                        # bank still frees in near-parallel with hh0's copies.
                        if on_act:
                            nc.scalar.copy(den_dst, cps[hh][64:65, 0:W])
                            nc.scalar.copy(ctx_dst, cps[hh][0:64, 0:W])
                        elif hh == 1:
                            nc.scalar.copy(den_dst, cps[hh][64:65, 0:W])
                            nc.vector.tensor_copy(ctx_dst, cps[hh][0:64, 0:W])
                        else:
                            nc.vector.tensor_copy(den_dst, cps[hh][64:65, 0:W])
                            nc.vector.tensor_copy(ctx_dst, cps[hh][0:64, 0:W])
                    if pt == 1:
                        tail_chunk(q0, W, last=False)
                    elif g == 3:
                        # the last chunk's pt0 half: normalize + project it
                        # mid-stream into its own partial (outU0) so the
                        # post-stream tail only handles the pt1 half
                        u0_tail(q0, W)

                pending = []
                final_pending = []
                tail_state = {}

                def tail_chunk(q0, W, last, cps=None, pt=None):
                    """Normalize q-range [q0, q0+W) + its output-projection
                    slice, split into small parts consumed one per stream
                    iteration so the PE burst never stalls the exp stream.

                    The LAST (half-width) chunk's chain runs post-stream from
                    a deep tail pool; everything else rides the stream."""
                    st = {}

                    if last:
                        # pt1-only: the pt0 half was normalized + projected
                        # mid-stream by u0_tail into its own partial. The
                        # post-stream chain runs in two pipelined 256-wide
                        # halves so the second half's reciprocal/normalize
                        # overlaps the first half's output projection.
                        HW2 = W // 2

                        def recip_h(h):
                            qh = q0 + h * HW2
                            # single-op approx recip (18 bits — the bf16
                            # broadcast path rounds to 8 anyway); full 128
                            # partitions: the custom DVE op is not trusted
                            # with a nonzero base partition
                            nc.vector.reciprocal_approx_fast(
                                out=rec[:, qh : qh + HW2],
                                in_=den[:, qh : qh + HW2],
                            )
                            nc.vector.tensor_copy(
                                rec_bf[:, qh : qh + HW2], rec[:, qh : qh + HW2]
                            )

                        def norm_h(h):
                            qh = q0 + h * HW2
                            bc = tail_state["pool"].tile(
                                [128, HW2], F32, tag="tl", name=f"bch{h}"
                            )
                            nc.tensor.matmul(
                                bc,
                                lhsT=sel[:, 1, :],
                                rhs=rec_bf[:, qh : qh + HW2],
                                start=True,
                                stop=True,
                            )
                            nc.vector.tensor_mul(
                                ctx_sb[:, 1, qh : qh + HW2],
                                ctx_sb[:, 1, qh : qh + HW2],
                                bc,
                            )

                        def out_h(h, mt2):
                            qh = q0 + h * HW2
                            key = f"ot{h}"
                            if mt2 == 0:
                                st[key] = outp.tile(
                                    [128, 8, HW2], BF16, tag="of", name=f"otf{h}"
                                )
                            ot = st[key]
                            tailp = tail_state["pool"]
                            for mt in (2 * mt2, 2 * mt2 + 1):
                                po = tailp.tile(
                                    [128, HW2], F32, tag="tl", name=f"pof{h}"
                                )
                                nc.tensor.matmul(
                                    po,
                                    lhsT=wo_sb[:, 1, mt * 128 : (mt + 1) * 128],
                                    rhs=ctx_sb[:, 1, qh : qh + HW2],
                                    start=True,
                                    stop=True,
                                )
                                if mt % 2 == 1:
                                    nc.scalar.add(
                                        ot[:, mt, :], po, bo_sb[:, mt : mt + 1]
                                    )
                                else:
                                    nc.vector.tensor_scalar_add(
                                        ot[:, mt, :], po, bo_sb[:, mt : mt + 1]
                                    )
                            # merged stores (4 mt rows each) on the idle sync
                            # ring: halves the serialized issuance train that
                            # otherwise dominates the post-compute tail, and
                            # keeps DMA issuances off the scalar queue where
                            # they would block the ACT adds
                            if mt2 in (1, 3):
                                m0 = 4 * (mt2 // 2)
                                nc.sync.dma_start(
                                    out=out_re[:, m0 : m0 + 4, qh : qh + HW2],
                                    in_=ot[:, m0 : m0 + 4, :],
                                )

                        def den_copy(h):
                            lo, hi = h * HW2, (h + 1) * HW2
                            for hh in range(2):
                                hd = 2 * pt + hh
                                dst = den[32 * hd : 32 * hd + 1, q0 + lo : q0 + hi]
                                if hh:
                                    nc.scalar.copy(dst, cps[hh][64:65, lo:hi])
                                else:
                                    nc.vector.tensor_copy(dst, cps[hh][64:65, lo:hi])

                        def ctx_copy(h):
                            lo, hi = h * HW2, (h + 1) * HW2
                            for hh in range(2):
                                dst = ctx_sb[
                                    hh * 64 : (hh + 1) * 64, pt, q0 + lo : q0 + hi
                                ]
                                if hh:
                                    nc.scalar.copy(dst, cps[hh][0:64, lo:hi])
                                else:
                                    nc.vector.tensor_copy(dst, cps[hh][0:64, lo:hi])

                        final_pending.append(lambda: den_copy(0))
                        final_pending.append(lambda: recip_h(0))
                        final_pending.append(lambda: ctx_copy(0))
                        final_pending.append(lambda: den_copy(1))
                        final_pending.append(lambda: norm_h(0))
                        final_pending.append(lambda: ctx_copy(1))
                        final_pending.append(lambda: recip_h(1))
                        for mt2 in range(2):
                            final_pending.append(lambda mt2=mt2: out_h(0, mt2))
                        final_pending.append(lambda: norm_h(1))
                        for mt2 in range(2, 4):
                            final_pending.append(lambda mt2=mt2: out_h(0, mt2))
                        for mt2 in range(4):
                            final_pending.append(lambda mt2=mt2: out_h(1, mt2))
                        return

                    def p_recip():
                        nc.vector.reciprocal_approx_fast(
                            out=rec[:, q0 : q0 + W],
                            in_=den[:, q0 : q0 + W],
                        )
                        nc.vector.tensor_copy(
                            rec_bf[:, q0 : q0 + W], rec[:, q0 : q0 + W]
                        )

                    def p_norm(pt):
                        bc = ps_x.tile([128, W], F32, tag="tp", name="bc")
                        nc.tensor.matmul(
                            bc,
                            lhsT=sel[:, pt, :],
                            rhs=rec_bf[:, q0 : q0 + W],
                            start=True,
                            stop=True,
                        )
                        nc.vector.tensor_mul(
                            ctx_sb[:, pt, q0 : q0 + W],
                            ctx_sb[:, pt, q0 : q0 + W],
                            bc,
                        )

                    def p_out(mt2):
                        if mt2 == 0:
                            st["ot"] = outp.tile(
                                [128, 8, W], BF16, tag="o", name="ot"
                            )
                        ot = st["ot"]
                        for mt in (2 * mt2, 2 * mt2 + 1):
                            po = ps_x.tile([128, W], F32, tag="tp", name="po")
                            for pt in range(2):
                                nc.tensor.matmul(
                                    po,
                                    lhsT=wo_sb[:, pt, mt * 128 : (mt + 1) * 128],
                                    rhs=ctx_sb[:, pt, q0 : q0 + W],
                                    start=(pt == 0),
                                    stop=(pt == 1),
                                )
                                # (accumulates over pt)
                            nc.vector.tensor_scalar_add(
                                ot[:, mt, :], po, bo_sb[:, mt : mt + 1]
                            )
                        # store each finished pair immediately: spreads DMA and
                        # shrinks the post-stream tail to the last pair only
                        nc.sync.dma_start(
                            out=out_re[:, 2 * mt2 : 2 * mt2 + 2, q0 : q0 + W],
                            in_=ot[:, 2 * mt2 : 2 * mt2 + 2, :],
                        )
                        if mt2 == 3:
                            st.pop("ot")

                    pending.append(p_recip)
                    pending.append(lambda: p_norm(0))
                    pending.append(lambda: p_norm(1))
                    for mt2 in range(4):
                        pending.append(lambda mt2=mt2: p_out(mt2))

                def u0_tail(q0, W):
                    """Normalize + project the LAST chunk's pt0 half into the
                    outU0 partial while the stream is still running on pt1
                    groups (its denominators are complete after group 3)."""
                    st = {}

                    def u_recip():
                        nc.vector.reciprocal_approx_fast(
                            out=rec[0:64, q0 : q0 + W],
                            in_=den[0:64, q0 : q0 + W],
                        )
                        nc.vector.tensor_copy(
                            rec_bf[0:64, q0 : q0 + W], rec[0:64, q0 : q0 + W]
                        )

                    def u_norm():
                        bc = ps_x.tile([128, W], F32, tag="tp", name="ubc")
                        nc.tensor.matmul(
                            bc,
                            lhsT=sel[:, 0, :],
                            rhs=rec_bf[:, q0 : q0 + W],
                            start=True,
                            stop=True,
                        )
                        nc.vector.tensor_mul(
                            ctx_sb[:, 0, q0 : q0 + W],
                            ctx_sb[:, 0, q0 : q0 + W],
                            bc,
                        )

                    def u_out(mt2):
                        if mt2 == 0:
                            st["ot"] = outp.tile(
                                [128, 8, W], BF16, tag="o", name="otu"
                            )
                        ot = st["ot"]
                        for mt in (2 * mt2, 2 * mt2 + 1):
                            po = ps_x.tile([128, W], F32, tag="tp", name="upo")
                            nc.tensor.matmul(
                                po,
                                lhsT=wo_sb[:, 0, mt * 128 : (mt + 1) * 128],
                                rhs=ctx_sb[:, 0, q0 : q0 + W],
                                start=True,
                                stop=True,
                            )
                            # bias rides the pt1 partial; this one is pure copy
                            nc.vector.tensor_copy(ot[:, mt, :], po)
                        nc.sync.dma_start(
                            out=out2_re[:, 2 * mt2 : 2 * mt2 + 2, :],
                            in_=ot[:, 2 * mt2 : 2 * mt2 + 2, :],
                        )
                        if mt2 == 3:
                            st.pop("ot")

                    pending.append(u_recip)
                    pending.append(u_norm)
                    for mt2 in range(4):
                        pending.append(lambda mt2=mt2: u_out(mt2))

                # ---------- the stream ----------
                # graduated ctx lag: large early (the first half is PE-bound
                # on projections; ctx shifts into pt1's slack), catching up
                # two-per-iteration once the projections are done, then a
                # short lag so the epilogue stays small.
                for fn in prod[-1]:
                    fn()
                scores(0)
                scores(1)
                ctx_next = 0
                for i in range(N_IT):
                    exp_step(i)
                    if i + 2 < N_IT:
                        scores(i + 2)
                    for fn in prod.get(i, []):
                        fn()
                    if i == 62:
                        # last ph1 use emitted: release its 2 PSUM banks so
                        # the tail pool can take them over
                        ph1_cm.__exit__(None, None, None)
                        ps_x_cm = tc.tile_pool(name="ps_x", bufs=2, space="PSUM")
                        ps_x = ps_x_cm.__enter__()
                    want = 0
                    if i >= 26:
                        want = 1
                    if 64 <= i and i % 2 == 0 and ctx_next < i - 4:
                        want = 2
                    if i >= 104 and ctx_next < i - 2:
                        want = 2
                    if i >= N_IT - 10 and not pending and ctx_next < i:
                        want = 2
                    for j in range(want):
                        if ctx_next <= i:
                            # don't start a new group's ctx (start=True, which
                            # waits on the previous group's PSUM-evacuation
                            # copies) back-to-back with the old group's last
                            # step — defer one iteration so the copies finish
                            if (
                                j == 1
                                and ctx_next % KT == 0
                                and i < N_IT - 12
                            ):
                                break
                            ctx_step(ctx_next)
                            ctx_next += 1
                    if pending and (i % 2 == 1 or i >= 108):
                        pending.pop(0)()
                        if i >= 112 and pending:
                            pending.pop(0)()
                while ctx_next < N_IT:
                    ctx_step(ctx_next)
                    ctx_next += 1
                    if pending:
                        pending.pop(0)()
                while pending:
                    pending.pop(0)()
                # score PSUM users are done: hand the sps/ps_x banks (6) to a
                # deep tail pool so the final out-proj pipelines freely. The
                # cps pool stays open — the last group's PSUM-evacuation
                # copies run interleaved inside the final tail chain.
                ps_x_cm.__exit__(None, None, None)
                ps_s_cm.__exit__(None, None, None)
                tailp_cm = tc.tile_pool(name="tailp", bufs=6, space="PSUM")
                tail_state["pool"] = tailp_cm.__enter__()
                while final_pending:
                    final_pending.pop(0)()
                tailp_cm.__exit__(None, None, None)
                ps_c_cm.__exit__(None, None, None)

    nc.compile()
    return nc

_PROGRAMS = {}


def _get_program(masked=False):
    if masked not in _PROGRAMS:
        _PROGRAMS[masked] = build_program(masked)
    return _PROGRAMS[masked]


def make_in_maps(inputs):
    hs = np.asarray(inputs["hidden_states"], dtype=np.float32)
    mask = np.asarray(inputs["attention_mask"], dtype=np.float32)
    Wq = np.asarray(inputs["Wq"], dtype=np.float32)
    bq = np.asarray(inputs["bq"], dtype=np.float32)
    Wk = np.asarray(inputs["Wk"], dtype=np.float32)
    bk = np.asarray(inputs["bk"], dtype=np.float32)
    Wv = np.asarray(inputs["Wv"], dtype=np.float32)
    bv = np.asarray(inputs["bv"], dtype=np.float32)
    Wo = np.asarray(inputs["Wo"], dtype=np.float32)
    bo = np.asarray(inputs["bo"], dtype=np.float32)

    # selector: sel[k, pt, m] = 1 iff k == 32*(2*pt + m//64) (same on all
    # cores; head h's reciprocal lives on partition 32*h)
    sel = np.zeros((128, 2, 128), np.float32)
    for pt in range(2):
        for hh in range(2):
            sel[32 * (2 * pt + hh), pt, hh * 64 : (hh + 1) * 64] = 1.0

    in_maps = []
    for c in range(NCORES):
        b = c // GROUPS
        g = c % GROUPS
        cols = slice(g * NHC, (g + 1) * NHC)
        negm = ((1.0 - mask[b]) * -10000.0).astype(np.float32)
        # bv folded through the output projection; bo added on one core/batch
        bo_eff = bv[cols] @ Wo[cols, :]
        if g == 0:
            bo_eff = bo_eff + bo
        bf = ml_dtypes.bfloat16
        wkc = Wk[:, cols]
        wqc = Wq[:, cols]
        wkq = np.concatenate(
            [wkc[:, 0:128], wqc[:, 0:128], wkc[:, 128:256], wqc[:, 128:256]],
            axis=1,
        )
        in_maps.append(
            {
                "xt": np.ascontiguousarray(hs[b].T.astype(bf)),
                "wkq": np.ascontiguousarray(wkq.astype(bf)),
                "wv": np.ascontiguousarray(Wv[:, cols].astype(bf)),
                "wo": np.ascontiguousarray(Wo[cols, :].astype(bf)),
                "small": np.ascontiguousarray(
                    np.concatenate(
                        [
                            bq[cols].reshape(2, 128).T,
                            bk[cols].reshape(2, 128).T,
                            negm.reshape(KT, 128).T,
                        ],
                        axis=1,
                    ).astype(np.float32)
                ),
                "bo": np.ascontiguousarray(
                    bo_eff.astype(np.float32).reshape(8, 128).T
                ),
                "sel": sel.astype(ml_dtypes.bfloat16),
            }
        )
    return in_maps


def gather_output(per_core_outs):
    out = np.empty((B, S, HID), dtype=np.float32)
    for b in range(B):
        acc = per_core_outs[b * GROUPS]["outT"].astype(np.float32)
        for g in range(1, GROUPS):
            acc = acc + per_core_outs[b * GROUPS + g]["outT"].astype(np.float32)
        # the last q-chunk's pt0 half lives in its own partial per core
        for g in range(GROUPS):
            acc[:, 1536:2048] += per_core_outs[b * GROUPS + g]["outU0"].astype(
                np.float32
            )
        out[b] = acc.T
    return out


def run(inputs, trace=False):
    mask = np.asarray(inputs["attention_mask"], dtype=np.float32)
    nc = _get_program(masked=bool((mask != 1.0).any()))
    in_maps = make_in_maps(inputs)
    res = run_bass_kernel_spmd(
        nc, in_maps, core_ids=list(range(NCORES)), trace=trace
    )
    return gather_output(res.results), res


def kernel(**inputs):
    out, _ = run(inputs, trace=False)
    return out



# revision 39
# speedup vs baseline: 1.0095x; 1.0070x over previous
"""Multi-head attention kernel for Trainium2, sharded over 8 NeuronCores.

Problem: B=2, S=2048, HIDDEN=1024, 16 heads, head_dim=64, fp32 in/out.

Sharding (data + tensor parallel per the hint): core c handles batch b=c//4
and head-group g=c%4 (4 heads = 256 hidden columns). QKV projections are
column-sharded, output projection row-sharded; each core returns a partial
out^T [1024, 2048] and the host sums the 4 partials per batch (the
row-parallel all-reduce) and transposes.

All matmul operands are bf16 (fp32 matmul runs at 1/4 PE rate: 2 half-speed
passes); accumulation is fp32 in PSUM and the final output is fp32. The
softmax denominator path (reciprocal) stays fp32.

Per-core device program (layouts chosen so no big intermediate needs a
transpose; only x is transposed once, on PE against a bf16 identity):
  x^T[k,s]   = PE transpose of x (cast to bf16 during the input DMA)
  Q^T/K^T    = Wq/Wk_cols^T @ x^T     -> [256, 2048] bf16, head-major
  V          = x @ Wv_cols            -> [2048, 256] natural, stored per
               kpos-tile with a ones column appended per head ([128, 4*65])
  scores^T   = K_h^T.T @ Q_h^T        -> [kpos 128, q 512] fp32 PSUM; the
               two heads of a 128-partition pair run row-packed (K=64,
               tile_position (0,0)/(64,0)) into two PSUM banks
  P^T        = exp(SCALE*scores^T + negmask[kpos])  (mask rides the ACT
               per-partition bias; scale is the free ACT affine) -> bf16
  ctx~^T     = [V_h | 1]^T @ P^T accumulated over kpos in PSUM -> [65, q];
               row 64 is the softmax denominator for free
  ctx^T     /= denom (fp32 reciprocal + PE broadcast via selector matmul)
  out^T     += Wo_rows^T @ ctx^T + bo_eff       (row-parallel partial, fp32)

bv is folded into bo_eff on the host (bv @ Wo_rows), bo added on one core
per batch group.
"""

import sys
import types

import ml_dtypes
import numpy as np

import concourse.bass as bass
import concourse.tile as tile
from concourse import bacc, mybir
from concourse.bass_utils import run_bass_kernel_spmd


def _install_ntff_hook_shim():
    """The agent image's antenv lacks axon_hooks, so trace=True dies on
    import. Recreate the module with the boot script's ctypes-based hook
    so NTFF profiling works."""
    if "antenv.axon_hooks" in sys.modules:
        return
    mod = types.ModuleType("antenv.axon_hooks")
    mod._hook = None

    def set_axon_ntff_profile_hook(h):
        mod._hook = h

    def get_axon_ntff_profile_hook():
        return mod._hook

    mod.set_axon_ntff_profile_hook = set_axon_ntff_profile_hook
    mod.get_axon_ntff_profile_hook = get_axon_ntff_profile_hook
    sys.modules["antenv.axon_hooks"] = mod
    try:
        from trn_agent_boot.trn_boot import _ntff_profile_via_ctypes

        mod._hook = _ntff_profile_via_ctypes("/opt/axon/libaxon_pjrt.so")
    except Exception:
        mod._hook = None


_install_ntff_hook_shim()

F32 = mybir.dt.float32
BF16 = mybir.dt.bfloat16

B = 2
S = 2048
HID = 1024
NH = 16  # total heads
DH = 64  # head dim
NCORES = 8
GROUPS = 4  # head groups (cores per batch)
NHC = 256  # hidden columns per core (4 heads * 64)
KT = 16  # kpos tiles of 128
SCALE = DH**-0.5

AF = mybir.ActivationFunctionType


def build_program(masked=False):
    nc = bacc.Bacc(
        "TRN2",
        target_bir_lowering=False,
        debug=False,
        enable_asserts=False,
        num_devices=NCORES,
    )

    # x is supplied pre-transposed (and bf16) by the host: x^T [HID, S]
    x_d = nc.dram_tensor("xt", [HID, S], BF16, kind="ExternalInput")
    # K and Q projection columns interleaved per pair so each pair's weights
    # arrive in ONE DMA: cols [wk0 | wq0 | wk1 | wq1], 128 each
    wkq_d = nc.dram_tensor("wkq", [HID, 2 * NHC], BF16, kind="ExternalInput")
    wv_d = nc.dram_tensor("wv", [HID, NHC], BF16, kind="ExternalInput")
    wo_d = nc.dram_tensor("wo", [NHC, HID], BF16, kind="ExternalInput")
    # small per-core constants batched into one DMA: bq | bk | negmask
    sm_d = nc.dram_tensor("small", [128, 4 + KT], F32, kind="ExternalInput")
    bo_d = nc.dram_tensor("bo", [128, 8], F32, kind="ExternalInput")
    sel_d = nc.dram_tensor("sel", [128, 2, 128], BF16, kind="ExternalInput")
    # bf16 partials: the host sums 4 per batch in fp32
    out_d = nc.dram_tensor("outT", [HID, S], BF16, kind="ExternalOutput")
    # extra partial for the last q-chunk: its pt0 half is projected
    # mid-stream (its denominators are ready after group 3), so the
    # post-stream tail only runs the pt1 half. Host adds this partial.
    out2_d = nc.dram_tensor("outU0", [HID, 512], BF16, kind="ExternalOutput")

    F32R = mybir.dt.float32r

    with tile.TileContext(nc) as tc:
        with tc.tile_pool(name="persist", bufs=1) as persist:
            small = persist.tile([128, 4 + KT], F32, tag="small")
            bq_sb = small[:, 0:2]
            bk_sb = small[:, 2:4]
            negm = small[:, 4 : 4 + KT]
            wo_sb = persist.tile([128, 2, HID], BF16, tag="wo")
            bo_sb = persist.tile([128, 8], F32, tag="bo")

            qt = persist.tile([128, 2, S], BF16, tag="qt")
            kt_sb = persist.tile([128, 2, S], BF16, tag="kt")
            vall = persist.tile([128, KT, 4 * 65], BF16, tag="vall")
            ctx_sb = persist.tile([128, 2, S], BF16, tag="ctx")
            # softmax denominators: head h on partition 32*h (engine APs must
            # start on a 32-partition boundary); unused partitions primed 1.0
            den = persist.tile([128, S], F32, tag="den")
            rec = persist.tile([128, S], F32, tag="rec")
            scr = persist.tile([128, S], F32, tag="scr")
            # small first slice on gpsimd so the PE prewarm isn't blocked
            # behind a 1.8us full-den DVE memset
            nc.gpsimd.memset(den[:, 0:128], 1.0)
            nc.gpsimd.memset(den[:, 128:], 1.0)
            # preload the ACT exp table set while input DMAs are in flight
            nc.scalar.activation(scr[:, 0:1], den[:, 0:1], AF.Exp)
            sel = persist.tile([128, 2, 128], BF16, tag="sel")
            rec_bf = persist.tile([128, S], BF16, tag="rec_bf")
            # the pt0-norm of the last chunk contracts sel over all 128
            # partitions; rows 64-127 of rec_bf are otherwise unwritten at
            # that point and must be finite
            nc.gpsimd.memset(rec_bf[64:128, 1536:2048], 1.0)

            # HAM prewarm: ~5us of dummy matmuls (on den, just memset)
            # so the free-running PE clock gate opens during the input-DMA
            # wait and the first real projections run at 2.4 GHz. Enough of
            # them that the PE stays warm until the first x/weight DMAs
            # land (~12.5us).
            warm_cm = tc.tile_pool(name="warm", bufs=1, space="PSUM")
            warm = warm_cm.__enter__()
            wps = warm.tile([128, 2, 128], F32, tag="wps")
            for wi in range(8):
                nc.tensor.matmul(
                    wps[:, wi % 2, :],
                    lhsT=den[:, 0:128],
                    rhs=den[:, 0:128],
                    start=True,
                    stop=True,
                )
            warm_cm.__exit__(None, None, None)

            out_re = out_d[:].rearrange("(a p) s -> p a s", p=128)
            out2_re = out2_d[:].rearrange("(a p) s -> p a s", p=128)

            # The whole kernel is emitted as one software-pipelined stream:
            # global attention iteration i = 0..127 (group g = i//16 =
            # (pair, q-chunk), t = i%16). exp(i) leads; scores run 2 ahead,
            # ctx LAG behind (so V production and ctx never block the exp
            # stream); x/K/Q/V production blocks are interleaved at fixed
            # stream positions. PSUM: sps 2x2 + cps 2x1 + ph1/ps_x 2 = 8.
            LAG = 10

            with tc.tile_pool(name="expp", bufs=28) as expp, \
                 tc.tile_pool(name="bcp", bufs=2) as bcp, \
                 tc.tile_pool(name="outp", bufs=2) as outp, \
                 tc.tile_pool(name="win", bufs=1) as win_p, \
                 tc.tile_pool(name="xtp", bufs=1) as xtp:

                # ps_c first: it must outlive ps_s/ps_x (LIFO pool stack) —
                # the last group's PSUM evacuation runs inside the final tail
                ps_c_cm = tc.tile_pool(name="ps_c", bufs=1, space="PSUM")
                ps_c = ps_c_cm.__enter__()
                ps_s_cm = tc.tile_pool(name="ps_s", bufs=2, space="PSUM")
                ps_s = ps_s_cm.__enter__()
                ph1_cm = tc.tile_pool(name="ph1", bufs=2, space="PSUM")
                ph1 = ph1_cm.__enter__()

                kq_sb = win_p.tile([128, 8, 2 * NHC], BF16, tag="wkq")
                wv_sb = win_p.tile([128, 8, NHC], BF16, tag="wv")

                def kq_dma(pt, eng=None, part=None):
                    # one DMA per pair for both K and Q weights; pair 0 rides
                    # the scalar HWDGE ring, parallel with xt on the sync
                    # ring, split K-cols-first so the K projection (which
                    # leads the stream) can start half a transfer earlier
                    lo = pt * 256 + (128 if part == 1 else 0)
                    hi = pt * 256 + (128 if part == 0 else 256)
                    (eng or nc.sync).dma_start(
                        out=kq_sb[:, :, lo:hi],
                        in_=wkq_d[:].rearrange("(a p) n -> p a n", p=128)[
                            :, :, lo:hi
                        ],
                    )
                xT = xtp.tile([128, 8, S], BF16, tag="xT")

                # ---------- production helpers ----------
                xt_re = x_d[:].rearrange("(a p) s -> p a s", p=128)

                def xt_dma(c2, kh=None, width=512, eng=None, kr=None):
                    s0 = c2 * 512
                    if kr is not None:
                        k0, k1 = kr
                    else:
                        k0, k1 = (0, 8) if kh is None else (4 * kh, 4 * kh + 4)
                    (eng or nc.sync).dma_start(
                        out=xT[:, k0:k1, s0 : s0 + width],
                        in_=xt_re[:, k0:k1, s0 : s0 + width],
                    )

                pp_half = {}

                def proj_kq(off, bsb, dst, pt, j4, half=None, kr=None):
                    """One K/Q projection chunk (weight cols at `off` in
                    kq_sb); half=0/1 emits 4 of the 8 accumulating matmuls so
                    a chunk can straddle two stream slots without a long PE
                    burst blocking the exp stream. kr=(lo,hi) emits an
                    arbitrary kj range (used at startup to chase the x
                    DMA pieces)."""
                    key = (off, j4)
                    if kr is not None:
                        lo, hi = kr
                    elif half == 1:
                        lo, hi = 4, 8
                    elif half == 0:
                        lo, hi = 0, 4
                    else:
                        lo, hi = 0, 8
                    if lo == 0:
                        pp = ph1.tile([128, 512], F32, tag="ph1", name="pp")
                        if hi < 8:
                            pp_half[key] = pp
                    else:
                        pp = pp_half[key] if hi < 8 else pp_half.pop(key)
                    for kj in range(lo, hi):
                        nc.tensor.matmul(
                            pp,
                            lhsT=kq_sb[:, kj, off : off + 128],
                            rhs=xT[:, kj, j4 * 512 : (j4 + 1) * 512],
                            start=(kj == 0),
                            stop=(kj == 7),
                        )
                    if hi == 8:
                        nc.vector.tensor_scalar_add(
                            dst[:, pt, j4 * 512 : (j4 + 1) * 512],
                            pp,
                            bsb[:, pt : pt + 1],
                        )

                def proj_v(mt):
                    pv = ph1.tile([128, NHC], F32, tag="ph1", name="pv")
                    for kj in range(8):
                        nc.tensor.matmul(
                            pv,
                            lhsT=xT[:, kj, mt * 128 : (mt + 1) * 128],
                            rhs=wv_sb[:, kj, :],
                            start=(kj == 0),
                            stop=(kj == 7),
                        )
                    v_slot = vall[:, mt, :].rearrange("p (h e) -> p h e", h=4)
                    nc.vector.tensor_copy(
                        v_slot[:, :, 0:64], pv.rearrange("p (h d) -> p h d", h=4)
                    )
                    nc.gpsimd.memset(v_slot[:, :, 64:65], 1.0)

                # production schedule: stream position -> emitters.
                # c2 blocks feed group-0 scores just in time; V[t] must land
                # before ctx(t) at stream t+LAG; Q0[j]/K1/Q1 feed later groups.
                def late_dmas():
                    nc.sync.dma_start(
                        out=wo_sb, in_=wo_d[:].rearrange("(a p) n -> p a n", p=128)
                    )
                    nc.sync.dma_start(out=sel, in_=sel_d[:])
                    nc.sync.dma_start(out=bo_sb, in_=bo_d[:])

                def wv_dma():
                    nc.sync.dma_start(
                        out=wv_sb, in_=wv_d[:].rearrange("(a p) n -> p a n", p=128)
                    )

                def small_dma():
                    # tiny; rides the gpsimd SWDGE queue so the two HWDGE
                    # rings stay dedicated to x / weights during startup
                    nc.gpsimd.dma_start(out=small, in_=sm_d[:])

                def pk(pt, j4, half=None, kr=None):
                    proj_kq(pt * 256, bk_sb, kt_sb, pt, j4, half, kr)

                def pq(pt, j4, half=None, kr=None):
                    proj_kq(pt * 256 + 128, bq_sb, qt, pt, j4, half, kr)


                prod = {
                    # x arrives in 2-kj pieces and the first K/Q chunks are
                    # emitted in matching kj quarters, so the PE starts on
                    # the first 256KB piece instead of waiting for the full
                    # 512KB half. K weights land before Q so K leads.
                    -1: [lambda: kq_dma(0, nc.scalar, part=0),
                         lambda: kq_dma(0, nc.scalar, part=1),
                         lambda: xt_dma(0, kr=(0, 2)),
                         lambda: xt_dma(0, kr=(2, 4)),
                         lambda: xt_dma(0, kr=(4, 6)),
                         lambda: xt_dma(0, kr=(6, 8)),
                         small_dma,
                         lambda: pk(0, 0, kr=(0, 2)),
                         lambda: pk(0, 0, kr=(2, 4)),
                         lambda: pq(0, 0, kr=(0, 2)),
                         lambda: pq(0, 0, kr=(2, 4)),
                         lambda: pk(0, 0, kr=(4, 6)),
                         lambda: pq(0, 0, kr=(4, 6)),
                         lambda: pk(0, 0, kr=(6, 8)),
                         lambda: pq(0, 0, kr=(6, 8))],
                    0: [lambda: xt_dma(1)],
                    1: [wv_dma, lambda: pk(0, 1)],
                    3: [lambda: xt_dma(2)],
                    4: [lambda: pk(0, 2)],
                    5: [lambda: xt_dma(3)],
                    6: [lambda: pk(0, 3)],
                    8: [lambda: proj_v(0)],
                    9: [lambda: proj_v(1)],
                    10: [lambda: pq(0, 1)],
                    11: [lambda: proj_v(2)],
                    12: [lambda: proj_v(3)],
                    13: [lambda: proj_v(4)],
                    14: [lambda: proj_v(5)],
                    15: [lambda: proj_v(6)],
                    16: [lambda: proj_v(7)],
                    17: [lambda: proj_v(8)],
                    18: [lambda: proj_v(9), lambda: kq_dma(1)],
                    19: [lambda: proj_v(10)],
                    20: [lambda: proj_v(11)],
                    21: [lambda: proj_v(12)],
                    22: [lambda: proj_v(13)],
                    23: [lambda: proj_v(14)],
                    24: [lambda: proj_v(15)],
                    26: [late_dmas],
                    27: [lambda: pq(0, 2, 0)],
                    29: [lambda: pq(0, 2, 1)],
                    33: [lambda: pq(0, 3, 0)],
                    35: [lambda: pq(0, 3, 1)],
                    36: [lambda: pk(1, 0, 0)],
                    38: [lambda: pk(1, 0, 1)],
                    39: [lambda: pk(1, 1, 0)],
                    41: [lambda: pk(1, 1, 1)],
                    42: [lambda: pk(1, 2, 0)],
                    44: [lambda: pk(1, 2, 1)],
                    45: [lambda: pk(1, 3, 0)],
                    47: [lambda: pk(1, 3, 1)],
                    48: [lambda: pq(1, 0, 0)],
                    50: [lambda: pq(1, 0, 1)],
                    51: [lambda: pq(1, 1, 0)],
                    53: [lambda: pq(1, 1, 1)],
                    54: [lambda: pq(1, 2, 0)],
                    56: [lambda: pq(1, 2, 1)],
                    57: [lambda: pq(1, 3, 0)],
                    58: [lambda: pq(1, 3, 1)],
                }

                # ---------- attention stream state ----------
                # groups of (pt, q0, width): the final 512-chunk is split
                # into two 256-wide half-groups so the post-stream serial
                # tail (normalize + out-proj of the very last data) is half
                # as long, and the first half's tail overlaps the stream.
                groups_tbl = [
                    (0, 0, 512), (0, 512, 512), (0, 1024, 512), (0, 1536, 512),
                    (1, 0, 512), (1, 512, 512), (1, 1024, 512), (1, 1536, 512),
                ]
                N_IT = 16 * len(groups_tbl)
                LAST_G = len(groups_tbl) - 1
                cps_by_group = {}
                eps = {}
                sps_q = {}
                ps_x = None  # opened after ph1 closes (bank handoff)

                def scores(k):
                    g, t = k // 16, k % 16
                    pt, q0, W = groups_tbl[g]
                    # tiles stay full-width (one whole PSUM bank per head) so
                    # accumulation zero-regions are never shared across groups
                    sps = ps_s.tile([128, 2, 512], F32, tag="s", name="sps")
                    for hh in range(2):
                        nc.tensor.matmul(
                            sps[:, hh, 0:W],
                            lhsT=kt_sb[
                                hh * 64 : (hh + 1) * 64, pt, t * 128 : (t + 1) * 128
                            ],
                            rhs=qt[hh * 64 : (hh + 1) * 64, pt, q0 : q0 + W],
                            start=True,
                            stop=True,
                            tile_position=(hh * 64, 0),
                        )
                    sps_q[k] = sps

                def exp_step(k):
                    g, t = k // 16, k % 16
                    W = groups_tbl[g][2]
                    ep = expp.tile([128, 2, W], BF16, tag="e", name="ep")
                    # the additive mask is identically zero for all-ones
                    # attention_mask (the spec'd fill); skip the per-partition
                    # bias AP read in that case
                    bias = negm[:, t : t + 1] if masked else 0.0
                    nc.scalar.activation(
                        ep,
                        sps_q.pop(k)[:, :, 0:W],
                        AF.Exp,
                        bias=bias,
                        scale=float(SCALE),
                    )
                    eps[k] = ep

                def ctx_step(k):
                    g, t = k // 16, k % 16
                    pt, q0, W = groups_tbl[g]
                    if t == 0:
                        cps_by_group[g] = [
                            ps_c.tile([65, 512], F32, tag=f"c{h}", name=f"cps{h}")
                            for h in range(2)
                        ]
                    cps = cps_by_group[g]
                    ep = eps.pop(k)
                    for hh in range(2):
                        c0 = (2 * pt + hh) * 65
                        nc.tensor.matmul(
                            cps[hh][:, 0:W],
                            lhsT=vall[:, t, c0 : c0 + 65],
                            rhs=ep[:, hh, :],
                            start=(t == 0),
                            stop=(t == KT - 1),
                        )
                    if t == KT - 1:
                        finish_group(g)

                def finish_group(g):
                    pt, q0, W = groups_tbl[g]
                    cps = cps_by_group.pop(g)
                    if g == LAST_G:
                        # the whole post-stream chain hangs off the copies:
                        # tail_chunk interleaves them in 256-wide halves with
                        # the reciprocal/normalize chain so the first half's
                        # reciprocal starts after ~1/4 of the copy work
                        tail_chunk(q0, W, last=True, cps=cps, pt=pt)
                        return
                    for hh in range(2):
                        # hh=1 copies ride ACT so both heads' PSUM banks free
                        # in parallel — the next group's first ctx matmul
                        # (start=True) blocks the in-order PE queue until
                        # they do. Groups whose copies would interleave with
                        # the final exps (which gate the whole tail), or that
                        # land where the exp stream is ACT-saturated, keep
                        # the big ctx copy on DVE.
                        on_act = hh == 1 and g <= 3
                        ctx_dst = ctx_sb[hh * 64 : (hh + 1) * 64, pt, q0 : q0 + W]
                        h = 2 * pt + hh
                        den_dst = den[32 * h : 32 * h + 1, q0 : q0 + W]
                        # den first: it is tiny and gates the reciprocal,
                        # while the big ctx copy only gates the later muls.
                        # For late groups only the tiny den copy rides ACT
                        # (the exp stream is ACT-saturated there) so the cps
                        # bank still frees in near-parallel with hh0's copies.
                        if on_act:
                            nc.scalar.copy(den_dst, cps[hh][64:65, 0:W])
                            nc.scalar.copy(ctx_dst, cps[hh][0:64, 0:W])
                        elif hh == 1:
                            nc.scalar.copy(den_dst, cps[hh][64:65, 0:W])
                            nc.vector.tensor_copy(ctx_dst, cps[hh][0:64, 0:W])
                        else:
                            nc.vector.tensor_copy(den_dst, cps[hh][64:65, 0:W])
                            nc.vector.tensor_copy(ctx_dst, cps[hh][0:64, 0:W])
                    if pt == 1:
                        tail_chunk(q0, W, last=False)
                    elif g == 3:
                        # the last chunk's pt0 half: normalize + project it
                        # mid-stream into its own partial (outU0) so the
                        # post-stream tail only handles the pt1 half
                        u0_tail(q0, W)

                pending = []
                final_pending = []
                tail_state = {}

                def tail_chunk(q0, W, last, cps=None, pt=None):
                    """Normalize q-range [q0, q0+W) + its output-projection
                    slice, split into small parts consumed one per stream
                    iteration so the PE burst never stalls the exp stream.

                    The LAST (half-width) chunk's chain runs post-stream from
                    a deep tail pool; everything else rides the stream."""
                    st = {}

                    if last:
                        # pt1-only: the pt0 half was normalized + projected
                        # mid-stream by u0_tail into its own partial. The
                        # post-stream chain runs in two pipelined 256-wide
                        # halves so the second half's reciprocal/normalize
                        # overlaps the first half's output projection.
                        HW2 = W // 2

                        def recip_h(h):
                            qh = q0 + h * HW2
                            # single-op approx recip (18 bits — the bf16
                            # broadcast path rounds to 8 anyway); full 128
                            # partitions: the custom DVE op is not trusted
                            # with a nonzero base partition
                            nc.vector.reciprocal_approx_fast(
                                out=rec[:, qh : qh + HW2],
                                in_=den[:, qh : qh + HW2],
                            )
                            nc.vector.tensor_copy(
                                rec_bf[:, qh : qh + HW2], rec[:, qh : qh + HW2]
                            )

                        def norm_h(h):
                            qh = q0 + h * HW2
                            bc = tail_state["pool"].tile(
                                [128, HW2], F32, tag="tl", name=f"bch{h}"
                            )
                            nc.tensor.matmul(
                                bc,
                                lhsT=sel[:, 1, :],
                                rhs=rec_bf[:, qh : qh + HW2],
                                start=True,
                                stop=True,
                            )
                            nc.vector.tensor_mul(
                                ctx_sb[:, 1, qh : qh + HW2],
                                ctx_sb[:, 1, qh : qh + HW2],
                                bc,
                            )

                        def out_h(h, mt2):
                            qh = q0 + h * HW2
                            key = f"ot{h}"
                            if mt2 == 0:
                                st[key] = outp.tile(
                                    [128, 8, HW2], BF16, tag="of", name=f"otf{h}"
                                )
                            ot = st[key]
                            tailp = tail_state["pool"]
                            for mt in (2 * mt2, 2 * mt2 + 1):
                                po = tailp.tile(
                                    [128, HW2], F32, tag="tl", name=f"pof{h}"
                                )
                                nc.tensor.matmul(
                                    po,
                                    lhsT=wo_sb[:, 1, mt * 128 : (mt + 1) * 128],
                                    rhs=ctx_sb[:, 1, qh : qh + HW2],
                                    start=True,
                                    stop=True,
                                )
                                if mt % 2 == 1:
                                    nc.scalar.add(
                                        ot[:, mt, :], po, bo_sb[:, mt : mt + 1]
                                    )
                                else:
                                    nc.vector.tensor_scalar_add(
                                        ot[:, mt, :], po, bo_sb[:, mt : mt + 1]
                                    )
                            # merged stores (4 mt rows each) on the idle sync
                            # ring: halves the serialized issuance train that
                            # otherwise dominates the post-compute tail, and
                            # keeps DMA issuances off the scalar queue where
                            # they would block the ACT adds
                            if mt2 in (1, 3):
                                m0 = 4 * (mt2 // 2)
                                nc.sync.dma_start(
                                    out=out_re[:, m0 : m0 + 4, qh : qh + HW2],
                                    in_=ot[:, m0 : m0 + 4, :],
                                )

                        def den_copy(h):
                            lo, hi = h * HW2, (h + 1) * HW2
                            for hh in range(2):
                                hd = 2 * pt + hh
                                dst = den[32 * hd : 32 * hd + 1, q0 + lo : q0 + hi]
                                if hh:
                                    nc.scalar.copy(dst, cps[hh][64:65, lo:hi])
                                else:
                                    nc.vector.tensor_copy(dst, cps[hh][64:65, lo:hi])

                        def ctx_copy(h):
                            lo, hi = h * HW2, (h + 1) * HW2
                            for hh in range(2):
                                dst = ctx_sb[
                                    hh * 64 : (hh + 1) * 64, pt, q0 + lo : q0 + hi
                                ]
                                if hh:
                                    nc.scalar.copy(dst, cps[hh][0:64, lo:hi])
                                else:
                                    nc.vector.tensor_copy(dst, cps[hh][0:64, lo:hi])

                        final_pending.append(lambda: den_copy(0))
                        final_pending.append(lambda: recip_h(0))
                        final_pending.append(lambda: ctx_copy(0))
                        final_pending.append(lambda: den_copy(1))
                        final_pending.append(lambda: norm_h(0))
                        final_pending.append(lambda: ctx_copy(1))
                        final_pending.append(lambda: recip_h(1))
                        for mt2 in range(2):
                            final_pending.append(lambda mt2=mt2: out_h(0, mt2))
                        final_pending.append(lambda: norm_h(1))
                        for mt2 in range(2, 4):
                            final_pending.append(lambda mt2=mt2: out_h(0, mt2))
                        for mt2 in range(4):
                            final_pending.append(lambda mt2=mt2: out_h(1, mt2))
                        return

                    def p_recip():
                        nc.vector.reciprocal_approx_fast(
                            out=rec[:, q0 : q0 + W],
                            in_=den[:, q0 : q0 + W],
                        )
                        nc.vector.tensor_copy(
                            rec_bf[:, q0 : q0 + W], rec[:, q0 : q0 + W]
                        )

                    def p_norm(pt):
                        bc = ps_x.tile([128, W], F32, tag="tp", name="bc")
                        nc.tensor.matmul(
                            bc,
                            lhsT=sel[:, pt, :],
                            rhs=rec_bf[:, q0 : q0 + W],
                            start=True,
                            stop=True,
                        )
                        nc.vector.tensor_mul(
                            ctx_sb[:, pt, q0 : q0 + W],
                            ctx_sb[:, pt, q0 : q0 + W],
                            bc,
                        )

                    def p_out(mt2):
                        if mt2 == 0:
                            st["ot"] = outp.tile(
                                [128, 8, W], BF16, tag="o", name="ot"
                            )
                        ot = st["ot"]
                        for mt in (2 * mt2, 2 * mt2 + 1):
                            po = ps_x.tile([128, W], F32, tag="tp", name="po")
                            for pt in range(2):
                                nc.tensor.matmul(
                                    po,
                                    lhsT=wo_sb[:, pt, mt * 128 : (mt + 1) * 128],
                                    rhs=ctx_sb[:, pt, q0 : q0 + W],
                                    start=(pt == 0),
                                    stop=(pt == 1),
                                )
                                # (accumulates over pt)
                            nc.vector.tensor_scalar_add(
                                ot[:, mt, :], po, bo_sb[:, mt : mt + 1]
                            )
                        # store each finished pair immediately: spreads DMA and
                        # shrinks the post-stream tail to the last pair only
                        nc.sync.dma_start(
                            out=out_re[:, 2 * mt2 : 2 * mt2 + 2, q0 : q0 + W],
                            in_=ot[:, 2 * mt2 : 2 * mt2 + 2, :],
                        )
                        if mt2 == 3:
                            st.pop("ot")

                    pending.append(p_recip)
                    pending.append(lambda: p_norm(0))
                    pending.append(lambda: p_norm(1))
                    for mt2 in range(4):
                        pending.append(lambda mt2=mt2: p_out(mt2))

                def u0_tail(q0, W):
                    """Normalize + project the LAST chunk's pt0 half into the
                    outU0 partial while the stream is still running on pt1
                    groups (its denominators are complete after group 3)."""
                    st = {}

                    def u_recip():
                        nc.vector.reciprocal_approx_fast(
                            out=rec[0:64, q0 : q0 + W],
                            in_=den[0:64, q0 : q0 + W],
                        )
                        nc.vector.tensor_copy(
                            rec_bf[0:64, q0 : q0 + W], rec[0:64, q0 : q0 + W]
                        )

                    def u_norm():
                        bc = ps_x.tile([128, W], F32, tag="tp", name="ubc")
                        nc.tensor.matmul(
                            bc,
                            lhsT=sel[:, 0, :],
                            rhs=rec_bf[:, q0 : q0 + W],
                            start=True,
                            stop=True,
                        )
                        nc.vector.tensor_mul(
                            ctx_sb[:, 0, q0 : q0 + W],
                            ctx_sb[:, 0, q0 : q0 + W],
                            bc,
                        )

                    def u_out(mt2):
                        if mt2 == 0:
                            st["ot"] = outp.tile(
                                [128, 8, W], BF16, tag="o", name="otu"
                            )
                        ot = st["ot"]
                        for mt in (2 * mt2, 2 * mt2 + 1):
                            po = ps_x.tile([128, W], F32, tag="tp", name="upo")
                            nc.tensor.matmul(
                                po,
                                lhsT=wo_sb[:, 0, mt * 128 : (mt + 1) * 128],
                                rhs=ctx_sb[:, 0, q0 : q0 + W],
                                start=True,
                                stop=True,
                            )
                            # bias rides the pt1 partial; this one is pure copy
                            nc.vector.tensor_copy(ot[:, mt, :], po)
                        nc.sync.dma_start(
                            out=out2_re[:, 2 * mt2 : 2 * mt2 + 2, :],
                            in_=ot[:, 2 * mt2 : 2 * mt2 + 2, :],
                        )
                        if mt2 == 3:
                            st.pop("ot")

                    pending.append(u_recip)
                    pending.append(u_norm)
                    for mt2 in range(4):
                        pending.append(lambda mt2=mt2: u_out(mt2))

                # ---------- the stream ----------
                # graduated ctx lag: large early (the first half is PE-bound
                # on projections; ctx shifts into pt1's slack), catching up
                # two-per-iteration once the projections are done, then a
                # short lag so the epilogue stays small.
                for fn in prod[-1]:
                    fn()
                scores(0)
                scores(1)
                ctx_next = 0
                for i in range(N_IT):
                    exp_step(i)
                    if i + 2 < N_IT:
                        scores(i + 2)
                    for fn in prod.get(i, []):
                        fn()
                    if i == 62:
                        # last ph1 use emitted: release its 2 PSUM banks so
                        # the tail pool can take them over
                        ph1_cm.__exit__(None, None, None)
                        ps_x_cm = tc.tile_pool(name="ps_x", bufs=2, space="PSUM")
                        ps_x = ps_x_cm.__enter__()
                    want = 0
                    if i >= 26:
                        want = 1
                    if 64 <= i and i % 2 == 0 and ctx_next < i - 4:
                        want = 2
                    if i >= 104 and ctx_next < i - 2:
                        want = 2
                    if i >= N_IT - 10 and not pending and ctx_next < i:
                        want = 2
                    for j in range(want):
                        if ctx_next <= i:
                            # don't start a new group's ctx (start=True, which
                            # waits on the previous group's PSUM-evacuation
                            # copies) back-to-back with the old group's last
                            # step — defer one iteration so the copies finish
                            if (
                                False
                                and j == 1
                                and ctx_next % KT == 0
                                and i < N_IT - 12
                            ):
                                break
                            ctx_step(ctx_next)
                            ctx_next += 1
                    if pending and (i % 2 == 1 or i >= 108):
                        pending.pop(0)()
                        if i >= 112 and pending:
                            pending.pop(0)()
                while ctx_next < N_IT:
                    ctx_step(ctx_next)
                    ctx_next += 1
                    if pending:
                        pending.pop(0)()
                while pending:
                    pending.pop(0)()
                # score PSUM users are done: hand the sps/ps_x banks (6) to a
                # deep tail pool so the final out-proj pipelines freely. The
                # cps pool stays open — the last group's PSUM-evacuation
                # copies run interleaved inside the final tail chain.
                ps_x_cm.__exit__(None, None, None)
                ps_s_cm.__exit__(None, None, None)
                tailp_cm = tc.tile_pool(name="tailp", bufs=6, space="PSUM")
                tail_state["pool"] = tailp_cm.__enter__()
                while final_pending:
                    final_pending.pop(0)()
                tailp_cm.__exit__(None, None, None)
                ps_c_cm.__exit__(None, None, None)

    nc.compile()
    return nc

_PROGRAMS = {}


def _get_program(masked=False):
    if masked not in _PROGRAMS:
        _PROGRAMS[masked] = build_program(masked)
    return _PROGRAMS[masked]


def make_in_maps(inputs):
    hs = np.asarray(inputs["hidden_states"], dtype=np.float32)
    mask = np.asarray(inputs["attention_mask"], dtype=np.float32)
    Wq = np.asarray(inputs["Wq"], dtype=np.float32)
    bq = np.asarray(inputs["bq"], dtype=np.float32)
    Wk = np.asarray(inputs["Wk"], dtype=np.float32)
    bk = np.asarray(inputs["bk"], dtype=np.float32)
    Wv = np.asarray(inputs["Wv"], dtype=np.float32)
    bv = np.asarray(inputs["bv"], dtype=np.float32)
    Wo = np.asarray(inputs["Wo"], dtype=np.float32)
    bo = np.asarray(inputs["bo"], dtype=np.float32)

    # selector: sel[k, pt, m] = 1 iff k == 32*(2*pt + m//64) (same on all
    # cores; head h's reciprocal lives on partition 32*h)
    sel = np.zeros((128, 2, 128), np.float32)
    for pt in range(2):
        for hh in range(2):
            sel[32 * (2 * pt + hh), pt, hh * 64 : (hh + 1) * 64] = 1.0

    in_maps = []
    for c in range(NCORES):
        b = c // GROUPS
        g = c % GROUPS
        cols = slice(g * NHC, (g + 1) * NHC)
        negm = ((1.0 - mask[b]) * -10000.0).astype(np.float32)
        # bv folded through the output projection; bo added on one core/batch
        bo_eff = bv[cols] @ Wo[cols, :]
        if g == 0:
            bo_eff = bo_eff + bo
        bf = ml_dtypes.bfloat16
        wkc = Wk[:, cols]
        wqc = Wq[:, cols]
        wkq = np.concatenate(
            [wkc[:, 0:128], wqc[:, 0:128], wkc[:, 128:256], wqc[:, 128:256]],
            axis=1,
        )
        in_maps.append(
            {
                "xt": np.ascontiguousarray(hs[b].T.astype(bf)),
                "wkq": np.ascontiguousarray(wkq.astype(bf)),
                "wv": np.ascontiguousarray(Wv[:, cols].astype(bf)),
                "wo": np.ascontiguousarray(Wo[cols, :].astype(bf)),
                "small": np.ascontiguousarray(
                    np.concatenate(
                        [
                            bq[cols].reshape(2, 128).T,
                            bk[cols].reshape(2, 128).T,
                            negm.reshape(KT, 128).T,
                        ],
                        axis=1,
                    ).astype(np.float32)
                ),
                "bo": np.ascontiguousarray(
                    bo_eff.astype(np.float32).reshape(8, 128).T
                ),
                "sel": sel.astype(ml_dtypes.bfloat16),
            }
        )
    return in_maps


def gather_output(per_core_outs):
    out = np.empty((B, S, HID), dtype=np.float32)
    for b in range(B):
        acc = per_core_outs[b * GROUPS]["outT"].astype(np.float32)
        for g in range(1, GROUPS):
            acc = acc + per_core_outs[b * GROUPS + g]["outT"].astype(np.float32)
        # the last q-chunk's pt0 half lives in its own partial per core
        for g in range(GROUPS):
            acc[:, 1536:2048] += per_core_outs[b * GROUPS + g]["outU0"].astype(
                np.float32
            )
        out[b] = acc.T
    return out


def run(inputs, trace=False):
    mask = np.asarray(inputs["attention_mask"], dtype=np.float32)
    nc = _get_program(masked=bool((mask != 1.0).any()))
    in_maps = make_in_maps(inputs)
    res = run_bass_kernel_spmd(
        nc, in_maps, core_ids=list(range(NCORES)), trace=trace
    )
    return gather_output(res.results), res


def kernel(**inputs):
    out, _ = run(inputs, trace=False)
    return out

